# revision 22
# baseline (speedup 1.0000x reference)
"""Multi-head causal attention (b=2, t=2048, d=1024, H=16, hd=64) on 8 TRN2 cores.

Sharding: tensor-parallel over heads - 2 heads per core. Each core projects
the full token stream against its 128-dim weight slice, runs causal
attention for its 2 heads, and multiplies by its Wo row slice into a
[4096, 1024] bf16 partial that the host sums (+bias).

v2 layout/dtype strategy (per-core):
  * x, weights, out in bf16 (halves DMA traffic); projections in bf16.
  * q/k evicted to fp8e4 and DMA-repacked [64, 2j, t] so QK^T runs as an
    fp8 DoubleRow matmul (2 rows/cycle).
  * exp(scores) written as fp8 into [128, (2 blocks) x 512] tiles; the
    attn*V matmul contracts a PAIR of 128-token k-blocks per instruction
    with DoubleRow (vn pair [128, 2, 65]; the 65th column of ones gives
    softmax denominators for free).
  * numerics guard: causal rows q<512 attend k<256 through a bf16/f32r
    path (fp8 error does not average out on short rows).
  * V transposed via DMA-transpose XBAR instead of PE transposes.
  * projection (A) work for later batches is interleaved into attention
    (B) emission to fill PE dependency stalls.
"""

import sys

for _p in ("/opt/trn_rl_repo",):
    if _p not in sys.path:
        sys.path.insert(0, _p)

import numpy as np
import ml_dtypes

import concourse.bass as bass
import concourse.tile as tile
from concourse import mybir
from concourse import bass_utils

F32 = mybir.dt.float32
F32R = mybir.dt.float32r
BF16 = mybir.dt.bfloat16
FP8 = mybir.dt.float8e4
DR = mybir.MatmulPerfMode.DoubleRow

P = 128          # partitions
B = 2            # batch
T = 2048         # seq len
NT = B * T       # 4096 tokens
DIN = 1024       # model dim
HD = 64          # head dim
NDC = DIN // P   # 8 d_in chunks
NSPB = T // 512  # 4 token stripes per batch
KBB = T // P     # 16 k-blocks per batch
NTT = NT // P    # 32 token tiles
VW = HD + 1      # vn block width (64 dims + ones col)

N_CORES = 8

import os
USE_DR_SCORES = os.environ.get('NO_DR_SCORES', '') == ''
USE_DR_CTX = os.environ.get('NO_DR_CTX', '') == ''
SKIP_B = os.environ.get('SKIP_B', '') != ''
SKIP_EXEMPT = os.environ.get('SKIP_EXEMPT', '') != ''


def _split_multi_waits(nc, max_waits=1):
    """walrus in this container caps sync waits per instruction; spill
    extra waits onto same-engine NoOps inserted right before."""
    uid = 0
    for fn in nc.m.functions:
        for blk in fn.blocks:
            insts = blk.instructions
            new_list = []
            changed = False
            for inst in insts:
                si = inst.sync_info
                ow = list(si.on_wait) if si is not None and si.on_wait else []
                if len(ow) > max_waits:
                    spill, keep = ow[:-max_waits], ow[-max_waits:]
                    for w in spill:
                        nop = mybir.InstNoOp(name=f"I-wsplit-{blk.name}-{uid}", ins=[], outs=[])
                        uid += 1
                        nop.engine = inst.engine
                        nop.sync_info = mybir.SyncInfo(on_wait=[w], on_update=[])
                        new_list.append(nop)
                    inst.sync_info = mybir.SyncInfo(
                        on_wait=keep,
                        on_update=list(si.on_update) if si.on_update else [],
                    )
                    changed = True
                new_list.append(inst)
            if changed:
                insts[:] = new_list


def build():
    nc = bass.Bass("TRN2", target_bir_lowering=False, debug=False, num_devices=N_CORES)
    xt = nc.dram_tensor("xt", [DIN, NT], BF16, kind="ExternalInput").ap()
    wq = nc.dram_tensor("wq", [DIN, P], BF16, kind="ExternalInput").ap()
    wk = nc.dram_tensor("wk", [DIN, P], BF16, kind="ExternalInput").ap()
    wv = nc.dram_tensor("wv", [DIN, P], BF16, kind="ExternalInput").ap()
    wo = nc.dram_tensor("wo", [P, DIN], BF16, kind="ExternalInput").ap()
    out = nc.dram_tensor("out", [NT, DIN], BF16, kind="ExternalOutput").ap()

    with tile.TileContext(nc) as tc:
        with (
            tc.tile_pool(name="const", bufs=1) as const,
            tc.tile_pool(name="persist", bufs=1) as persist,
            tc.tile_pool(name="vstage_p", bufs=3) as vstage_p,
            tc.tile_pool(name="vtr_p", bufs=3) as vtr_p,
            tc.tile_pool(name="e8_p", bufs=4) as e8_p,
            tc.tile_pool(name="e32_p", bufs=2) as e32_p,
            tc.tile_pool(name="r_p", bufs=2) as r_p,
            tc.tile_pool(name="osb_p", bufs=2) as osb_p,
            tc.tile_pool(name="psA", bufs=1, space="PSUM") as psA,
            tc.tile_pool(name="psB", bufs=1, space="PSUM") as psB,
        ):
            # ---- constants / weights ----
            # head-selector for the rb broadcast: row 0 -> partitions 0:64,
            # row 64 -> partitions 64:128 (engine writes are legal at
            # partition bases 0 mod 32 only)
            hsel65 = const.tile([65, P], F32)
            nc.gpsimd.memset(hsel65[:], 0.0)
            nc.gpsimd.memset(hsel65[0:1, 0:64], 1.0)
            nc.gpsimd.memset(hsel65[64:65, 64:128], 1.0)


            wq_sb = const.tile([P, NDC * P], BF16)
            wk_sb = const.tile([P, NDC * P], BF16)
            wv_sb = const.tile([P, NDC * P], BF16)
            for w_dram, w_sb in ((wq, wq_sb), (wk, wk_sb), (wv, wv_sb)):
                nc.sync.dma_start(
                    w_sb[:].rearrange("p (c n) -> p c n", c=NDC),
                    w_dram.rearrange("(c p) n -> p c n", c=NDC),
                )
            wo_sb = const.tile([P, DIN], BF16)
            nc.sync.dma_start(wo_sb[:], wo)

            # x resident in SBUF: [p, (chunk c, token t)]
            xt_sb = const.tile([P, NDC * NT], BF16)
            for tq in range(8):
                nc.sync.dma_start(
                    xt_sb[:].rearrange("p (c n) -> p c n", c=NDC)[:, :, tq * 512:(tq + 1) * 512],
                    xt.rearrange("(c p) n -> p c n", c=NDC)[:, :, tq * 512:(tq + 1) * 512],
                )

            # ---- persistent activations ----
            qT8 = persist.tile([P, NT], FP8)   # rows: h0 dims 0-63, h1 64-127
            kT8 = persist.tile([P, NT], FP8)
            q8 = persist.tile([64, 2 * NT], FP8)   # [32h+d, (j, t)]; dim = 32j+d of head h
            k8 = persist.tile([64, 2 * NT], FP8)
            qbf = persist.tile([P, B * 512], BF16)  # stripe 0 of each batch
            kbf = persist.tile([P, B * 256], BF16)  # blocks 0-1 of each batch
            # vn blocks padded to 128 cols: [0:64] V dims, col 64 ones
            # (softmax denominators ride along), cols 65:128 zeros (dual-fp8
            # ldweights requires 32-aligned tile strides and M in {32,64,128})
            vn8 = [persist.tile([P, NTT * P], FP8, name=f"vn8_h{h}") for h in range(2)]
            vn32 = [persist.tile([P, B * 2 * P], F32R, name=f"vn32_h{h}") for h in range(2)]
            ctxT = persist.tile([P, NT], BF16)
            # reciprocal landing pad: rows 0 and 64 only; rows 1-63/65-127 stay
            # zero so the hsel65 broadcast matmul contracts clean zeros
            rinv2 = persist.tile([P, 2 * 512], F32)
            nc.gpsimd.memset(rinv2[:], 0.0)

            # ones/zeros pad columns of vn tiles. fp8 memset is legal;
            # f32r memset is not -> stage f32 and copy.
            for h in range(2):
                v8v = vn8[h][:].rearrange("p (t w) -> p t w", w=P)
                nc.gpsimd.memset(v8v[:, :, HD:HD + 1], 1.0)
                nc.gpsimd.memset(v8v[:, :, HD + 1:P], 0.0)
            z64 = const.tile([P, P - HD], F32)
            nc.gpsimd.memset(z64[:], 0.0)
            nc.gpsimd.memset(z64[:, 0:1], 1.0)
            for h in range(2):
                for blk in range(B * 2):
                    nc.vector.tensor_copy(
                        vn32[h][:, blk * P + HD:(blk + 1) * P], z64[:])

            q8v = q8[:].rearrange("p (j t) -> p j t", j=2)
            k8v = k8[:].rearrange("p (j t) -> p j t", j=2)

            # ---------------- A phase: projections ----------------
            def emit_proj_stripe(b, sl):
                s = b * NSPB + sl
                col = s * 512
                for w_sb, outs in (
                    (wq_sb, "q"), (wk_sb, "k"), (wv_sb, "v"),
                ):
                    ps = psA.tile([P, 512], F32, name=f"pp_{s}_{outs}", tag="pp", bufs=1)
                    for c in range(NDC):
                        yield_now = (c % 3 == 2)
                        nc.tensor.matmul(
                            ps[:], w_sb[:, c * P:(c + 1) * P],
                            xt_sb[:, c * NT + col: c * NT + col + 512],
                            start=(c == 0), stop=(c == NDC - 1),
                        )
                        if yield_now:
                            yield
                    if outs == "q":
                        nc.vector.tensor_copy(qT8[:, col:col + 512], ps[:])
                        if sl == 0:
                            nc.vector.tensor_copy(qbf[:, b * 512:(b + 1) * 512], ps[:])
                    elif outs == "k":
                        nc.vector.tensor_copy(kT8[:, col:col + 512], ps[:])
                        if sl == 0:
                            nc.vector.tensor_copy(kbf[:, b * 256:(b + 1) * 256], ps[:, 0:256])
                    else:
                        vstage = vstage_p.tile([P, 512], BF16, name="vstage")
                        nc.vector.tensor_copy(vstage[:], ps[:])
                        vtr = vtr_p.tile([P, 512], BF16, name="vtr")
                        vtrv = vtr[:].rearrange("p (i d) -> p i d", i=4)
                        nc.sync.dma_start(vtrv, vstage[:], transpose=True)
                        # vtr[p, i, d] = v[dim d, token col + 128i + p]
                        tt0 = s * 4
                        for h in range(2):
                            nc.vector.tensor_copy(
                                vn8[h][:].rearrange("p (t w) -> p t w", w=P)[:, tt0:tt0 + 4, 0:HD],
                                vtrv[:, :, h * HD:(h + 1) * HD],
                            )
                        if sl == 0:
                            for h in range(2):
                                nc.vector.tensor_copy(
                                    vn32[h][:].rearrange("p (t w) -> p t w", w=P)[:, b * 2:b * 2 + 2, 0:HD],
                                    vtrv[:, 0:2, h * HD:(h + 1) * HD],
                                )
                    yield

            def emit_repack(b, half):
                cols = slice(b * T + half * 1024, b * T + half * 1024 + 1024)
                for srcT, dst in ((qT8, q8), (kT8, k8)):
                    dstv = dst[:].rearrange("p (j t) -> p j t", j=2)
                    for h in range(2):
                        for j in range(2):
                            nc.sync.dma_start(
                                dstv[32 * h:32 * h + 32, j, cols],
                                srcT[64 * h + 32 * j: 64 * h + 32 * j + 32, cols],
                            )

            # A-work generator: stripes + repacks in dependency order.
            # Progress markers count completed repacks.
            progress = [0]

            def gen_A():
                plan = [
                    ("S", 0, 0), ("S", 0, 1), ("R", 0, 0),
                    ("S", 0, 2), ("S", 0, 3), ("R", 0, 1),
                    ("S", 1, 0), ("S", 1, 1), ("R", 1, 0),
                    ("S", 1, 2), ("S", 1, 3), ("R", 1, 1),
                ]
                for kind, b, i in plan:
                    if kind == "S":
                        yield from emit_proj_stripe(b, i)
                    else:
                        emit_repack(b, i)
                        progress[0] += 1
                        yield

            ga = gen_A()

            def drain_until(n_repacks):
                while progress[0] < n_repacks:
                    if next(ga, "DONE") == "DONE":
                        break

            from collections import deque
            pending = deque()

            pace = [0]

            def filler():
                while pending:
                    try:
                        next(pending[0])
                        return
                    except StopIteration:
                        pending.popleft()
                pace[0] += 1
                next(ga, None)

            # ---------------- B phase: attention + out-proj ----------------
            def cstart(m):
                return 0 if m < 0 else min(128 * m, 384)

            def emit_attn_qs(b, qs):
                qoff = b * T + qs * 512
                npair = 2 * qs + 2
                ctx_h = [
                    psB.tile([P, 512], F32, name=f"ctx_{b}{qs}{h}", tag="ctx", bufs=2)
                    for h in range(2)
                ]
                for prn, pr in enumerate(range(npair)):
                    kb0 = 2 * pr
                    ktt0 = b * KBB + kb0
                    m0 = kb0 - 4 * qs
                    m1 = m0 + 1
                    c_0, c_1 = cstart(m0), cstart(m1)
                    cp = c_0
                    exempt = (qs == 0 and pr == 0) and not SKIP_EXEMPT
                    es = []
                    for h in range(2):
                        ctx = ctx_h[h]
                        sp2 = psB.tile([P, 1024], F32, name=f"sp2_{b}{qs}{h}{pr}", tag="sp2", bufs=2)
                        if exempt:
                            for j, cj in ((0, c_0), (1, c_1)):
                                nc.tensor.matmul(
                                    sp2[:, j * 512 + cj:(j + 1) * 512],
                                    kbf[64 * h:64 * h + 64, b * 256 + j * 128: b * 256 + (j + 1) * 128],
                                    qbf[64 * h:64 * h + 64, b * 512 + cj:(b + 1) * 512],
                                    start=True, stop=True,
                                )
                        else:
                            for j, cj in ((0, c_0), (1, c_1)):
                                ktt = ktt0 + j
                                nc.tensor.matmul(
                                    sp2[:, j * 512 + cj:(j + 1) * 512],
                                    k8v[32 * h:32 * h + 32, :, ktt * P:(ktt + 1) * P],
                                    q8v[32 * h:32 * h + 32, :, qoff + cj:qoff + 512],
                                    start=True, stop=True, perf_mode=DR,
                                )
                        filler()
                        if exempt:
                            e = e32_p.tile([P, 1024], F32R, name="e32")
                        else:
                            e = e8_p.tile([P, 1024], FP8, name="e8")
                        es.append(e)
                        # exp each block over its exact live range
                        if c_0 == c_1:
                            nc.scalar.activation(
                                e[:, c_0:1024], sp2[:, c_0:1024],
                                mybir.ActivationFunctionType.Exp, scale=0.125)
                        else:
                            nc.scalar.activation(
                                e[:, c_0:512], sp2[:, c_0:512],
                                mybir.ActivationFunctionType.Exp, scale=0.125)
                            nc.scalar.activation(
                                e[:, 512 + c_1:1024], sp2[:, 512 + c_1:1024],
                                mybir.ActivationFunctionType.Exp, scale=0.125)
                        if m0 >= 0:
                            # one affine over both blocks: keep where
                            # (cp+i) - 128*(m0+j) - p >= 0; also zeroes the
                            # un-exp'd garbage cols [cp, cstart(m)) of each blk
                            ev = e[:].rearrange("p (j n) -> p j n", j=2)[:, :, cp:512]
                            nc.gpsimd.affine_select(
                                out=ev, in_=ev,
                                compare_op=mybir.AluOpType.is_ge,
                                fill=0.0, base=cp - 128 * m0,
                                pattern=[[-128, 2], [1, 512 - cp]],
                                channel_multiplier=-1,
                            )
                        elif m1 >= 0:
                            lo = 512 + cp
                            hi = 512 + min(128 * m1 + 128, 512)
                            nc.gpsimd.affine_select(
                                out=e[:, lo:hi], in_=e[:, lo:hi],
                                compare_op=mybir.AluOpType.is_ge,
                                fill=0.0, base=cp - 128 * m1,
                                pattern=[[1, hi - lo]],
                                channel_multiplier=-1,
                            )
                    st0 = (prn == 0)
                    stN = (prn == npair - 1)
                    for h in range(2):
                        ctx, e = ctx_h[h], es[h]
                        if exempt:
                            l0 = b * 2
                            nc.tensor.matmul(
                                ctx[:, c_0:512], vn32[h][:, l0 * P:(l0 + 1) * P],
                                e[:, c_0:512], start=st0, stop=False,
                            )
                            nc.tensor.matmul(
                                ctx[:, c_1:512], vn32[h][:, (l0 + 1) * P:(l0 + 2) * P],
                                e[:, 512 + c_1:1024], start=False, stop=stN,
                            )
                        else:
                            # masked cols of block j=1 in [cp, c_1) were zeroed
                            # by affine_select; block j=0 live from cp=c_0
                            nc.tensor.matmul(
                                ctx[:, cp:512],
                                vn8[h][:].rearrange("p (t w) -> p t w", w=P)[:, ktt0:ktt0 + 2, :],
                                e[:].rearrange("p (j n) -> p j n", j=2)[:, :, cp:512],
                                start=st0, stop=stN, perf_mode=DR,
                            )
                        filler()

                # ---- epilogue head: free the ctx psum slots quickly ----
                off = ((b * NSPB + qs) % 2) * 512
                with nc.allow_low_precision(reason="f32r softmax denominators"):
                    nc.vector.reciprocal(rinv2[0:1, off:off + 512], ctx_h[0][64:65, :])
                    nc.vector.reciprocal(rinv2[64:65, off:off + 512], ctx_h[1][64:65, :])
                ctxU = []
                for h in range(2):
                    u = r_p.tile([64, 512], BF16, name=f"ctxU{h}", tag=f"ctxU{h}")
                    nc.vector.tensor_copy(u[:], ctx_h[h][0:64, :])
                    ctxU.append(u)

                def tail():
                    rb = psB.tile([P, 512], F32, name=f"rb_{b}{qs}", tag="op", bufs=1)
                    nc.tensor.matmul(rb[:], hsel65[:], rinv2[0:65, off:off + 512],
                                     start=True, stop=True)
                    yield
                    for h in range(2):
                        nc.vector.tensor_mul(
                            ctxT[64 * h:64 * h + 64, qoff:qoff + 512],
                            ctxU[h][:], rb[64 * h:64 * h + 64, :])
                    yield
                    osb = osb_p.tile([P, 4096], BF16, name="osb")
                    for i in range(4):
                        coff = qoff + i * P
                        for half in range(2):
                            op = psB.tile([P, 512], F32, name=f"op_{b}{qs}{i}{half}",
                                          tag="op", bufs=1)
                            nc.tensor.matmul(
                                op[:], ctxT[:, coff:coff + P],
                                wo_sb[:, half * 512:(half + 1) * 512],
                                start=True, stop=True)
                            nc.vector.tensor_copy(
                                osb[:, i * 1024 + half * 512:i * 1024 + (half + 1) * 512],
                                op[:])
                            yield
                    nc.sync.dma_start(
                        out[qoff:qoff + 512, :].rearrange("(i p) d -> p i d", p=P),
                        osb[:].rearrange("p (i d) -> p i d", i=4),
                    )

                pending.append(tail())

            if SKIP_B:
                drain_until(99)
                # dummy out write so the output tensor has a writer
                dumb = osb_p.tile([P, 4096], BF16, name="dumb")
                nc.vector.tensor_copy(dumb[:, 0:512], qbf[:, 0:512])
                nc.sync.dma_start(
                    out[0:512, :].rearrange("(i p) d -> p i d", p=P),
                    dumb[:].rearrange("p (i d) -> p i d", i=4))
            else:
                for b in range(B):
                    for qs in range(NSPB):
                        drain_until(b * 2 + qs // 2 + 1)
                        emit_attn_qs(b, qs)
                drain_until(99)
                while pending:
                    for _ in pending.popleft():
                        pass

    _split_multi_waits(nc)
    return nc


_NC_CACHE = None


def _get_nc():
    global _NC_CACHE
    if _NC_CACHE is None:
        _NC_CACHE = build()
    return _NC_CACHE


def make_in_maps(x, Wq, Wk, Wv, Wo):
    xt = np.ascontiguousarray(
        np.asarray(x, dtype=np.float32).reshape(NT, DIN).T
    ).astype(ml_dtypes.bfloat16)
    in_maps = []
    for c in range(N_CORES):
        sl = slice(c * P, (c + 1) * P)
        in_maps.append({
            "xt": xt,
            "wq": np.ascontiguousarray(np.asarray(Wq, dtype=np.float32)[:, sl]).astype(ml_dtypes.bfloat16),
            "wk": np.ascontiguousarray(np.asarray(Wk, dtype=np.float32)[:, sl]).astype(ml_dtypes.bfloat16),
            "wv": np.ascontiguousarray(np.asarray(Wv, dtype=np.float32)[:, sl]).astype(ml_dtypes.bfloat16),
            "wo": np.ascontiguousarray(np.asarray(Wo, dtype=np.float32)[sl, :]).astype(ml_dtypes.bfloat16),
        })
    return in_maps


def kernel(x, Wq, Wk, Wv, Wo, bo):
    nc = _get_nc()
    in_maps = make_in_maps(x, Wq, Wk, Wv, Wo)
    res = bass_utils.run_bass_kernel_spmd(
        nc, in_maps, core_ids=list(range(N_CORES)), trace=False
    )
    acc = np.zeros((NT, DIN), dtype=np.float64)
    for r in res.results:
        acc += np.asarray(r["out"]).astype(np.float64)
    acc += np.asarray(bo, dtype=np.float64)[None, :]
    return acc.astype(np.float32).reshape(B, T, DIN)


if __name__ == "__main__":
    rng = np.random.default_rng(0)
    x = rng.standard_normal((B, T, DIN)).astype(np.float32)
    Wq = rng.standard_normal((DIN, DIN)).astype(np.float32) * 0.02
    Wk = rng.standard_normal((DIN, DIN)).astype(np.float32) * 0.02
    Wv = rng.standard_normal((DIN, DIN)).astype(np.float32) * 0.02
    Wo = rng.standard_normal((DIN, DIN)).astype(np.float32) * 0.02
    bo = np.zeros(DIN, dtype=np.float32)
    out = kernel(x=x, Wq=Wq, Wk=Wk, Wv=Wv, Wo=Wo, bo=bo)
    print("out", out.shape, out.dtype, float(np.abs(out).max()))


# revision 23
# speedup vs baseline: 1.0530x; 1.0530x over previous
"""Multi-head causal attention (b=2, t=2048, d=1024, H=16, hd=64) on 8 TRN2 cores.

Sharding: tensor-parallel over heads - 2 heads per core. Each core projects
the full token stream against its 128-dim weight slice, runs causal
attention for its 2 heads, and multiplies by its Wo row slice into a
[4096, 1024] bf16 partial that the host sums (+bias).

v2 layout/dtype strategy (per-core):
  * x, weights, out in bf16 (halves DMA traffic); projections in bf16.
  * q/k evicted to fp8e4 and DMA-repacked [64, 2j, t] so QK^T runs as an
    fp8 DoubleRow matmul (2 rows/cycle).
  * exp(scores) written as fp8 into [128, (2 blocks) x 512] tiles; the
    attn*V matmul contracts a PAIR of 128-token k-blocks per instruction
    with DoubleRow (vn pair [128, 2, 65]; the 65th column of ones gives
    softmax denominators for free).
  * numerics guard: causal rows q<512 attend k<256 through a bf16/f32r
    path (fp8 error does not average out on short rows).
  * V transposed via DMA-transpose XBAR instead of PE transposes.
  * projection (A) work for later batches is interleaved into attention
    (B) emission to fill PE dependency stalls.
"""

import sys

for _p in ("/opt/trn_rl_repo",):
    if _p not in sys.path:
        sys.path.insert(0, _p)

import numpy as np
import ml_dtypes

import concourse.bass as bass
import concourse.tile as tile
from concourse import mybir
from concourse import bass_utils

F32 = mybir.dt.float32
F32R = mybir.dt.float32r
BF16 = mybir.dt.bfloat16
FP8 = mybir.dt.float8e4
DR = mybir.MatmulPerfMode.DoubleRow

P = 128          # partitions
B = 2            # batch
T = 2048         # seq len
NT = B * T       # 4096 tokens
DIN = 1024       # model dim
HD = 64          # head dim
NDC = DIN // P   # 8 d_in chunks
NSPB = T // 512  # 4 token stripes per batch
KBB = T // P     # 16 k-blocks per batch
NTT = NT // P    # 32 token tiles
VW = HD + 1      # vn block width (64 dims + ones col)

N_CORES = 8

import os
USE_DR_SCORES = os.environ.get('NO_DR_SCORES', '') == ''
USE_DR_CTX = os.environ.get('NO_DR_CTX', '') == ''
SKIP_B = os.environ.get('SKIP_B', '') != ''
SKIP_EXEMPT = os.environ.get('SKIP_EXEMPT', '') != ''


def _split_multi_waits(nc, max_waits=1):
    """walrus in this container caps sync waits per instruction; spill
    extra waits onto same-engine NoOps inserted right before."""
    uid = 0
    for fn in nc.m.functions:
        for blk in fn.blocks:
            insts = blk.instructions
            new_list = []
            changed = False
            for inst in insts:
                si = inst.sync_info
                ow = list(si.on_wait) if si is not None and si.on_wait else []
                if len(ow) > max_waits:
                    spill, keep = ow[:-max_waits], ow[-max_waits:]
                    for w in spill:
                        nop = mybir.InstNoOp(name=f"I-wsplit-{blk.name}-{uid}", ins=[], outs=[])
                        uid += 1
                        nop.engine = inst.engine
                        nop.sync_info = mybir.SyncInfo(on_wait=[w], on_update=[])
                        new_list.append(nop)
                    inst.sync_info = mybir.SyncInfo(
                        on_wait=keep,
                        on_update=list(si.on_update) if si.on_update else [],
                    )
                    changed = True
                new_list.append(inst)
            if changed:
                insts[:] = new_list


def build():
    nc = bass.Bass("TRN2", target_bir_lowering=False, debug=False, num_devices=N_CORES)
    xt = nc.dram_tensor("xt", [DIN, NT], BF16, kind="ExternalInput").ap()
    wq = nc.dram_tensor("wq", [DIN, P], BF16, kind="ExternalInput").ap()
    wk = nc.dram_tensor("wk", [DIN, P], BF16, kind="ExternalInput").ap()
    wv = nc.dram_tensor("wv", [DIN, P], BF16, kind="ExternalInput").ap()
    wo = nc.dram_tensor("wo", [P, DIN], BF16, kind="ExternalInput").ap()
    out = nc.dram_tensor("out", [NT, DIN], BF16, kind="ExternalOutput").ap()

    with tile.TileContext(nc) as tc:
        with (
            tc.tile_pool(name="const", bufs=1) as const,
            tc.tile_pool(name="persist", bufs=1) as persist,
            tc.tile_pool(name="vstage_p", bufs=3) as vstage_p,
            tc.tile_pool(name="vtr_p", bufs=3) as vtr_p,
            tc.tile_pool(name="e8_p", bufs=4) as e8_p,
            tc.tile_pool(name="e32_p", bufs=2) as e32_p,
            tc.tile_pool(name="r_p", bufs=2) as r_p,
            tc.tile_pool(name="osb_p", bufs=2) as osb_p,
            tc.tile_pool(name="psA", bufs=1, space="PSUM") as psA,
            tc.tile_pool(name="psB", bufs=1, space="PSUM") as psB,
        ):
            # ---- constants / weights ----
            # head-selector for the rb broadcast: row 0 -> partitions 0:64,
            # row 64 -> partitions 64:128 (engine writes are legal at
            # partition bases 0 mod 32 only)
            hsel65 = const.tile([65, P], F32)
            nc.gpsimd.memset(hsel65[:], 0.0)
            nc.gpsimd.memset(hsel65[0:1, 0:64], 1.0)
            nc.gpsimd.memset(hsel65[64:65, 64:128], 1.0)


            wq_sb = const.tile([P, NDC * P], BF16)
            wk_sb = const.tile([P, NDC * P], BF16)
            wv_sb = const.tile([P, NDC * P], BF16)
            for w_dram, w_sb in ((wq, wq_sb), (wk, wk_sb), (wv, wv_sb)):
                nc.sync.dma_start(
                    w_sb[:].rearrange("p (c n) -> p c n", c=NDC),
                    w_dram.rearrange("(c p) n -> p c n", c=NDC),
                )
            wo_sb = const.tile([P, DIN], BF16)
            nc.sync.dma_start(wo_sb[:], wo)

            # x resident in SBUF: [p, (chunk c, token t)]
            xt_sb = const.tile([P, NDC * NT], BF16)
            for tq in range(8):
                nc.sync.dma_start(
                    xt_sb[:].rearrange("p (c n) -> p c n", c=NDC)[:, :, tq * 512:(tq + 1) * 512],
                    xt.rearrange("(c p) n -> p c n", c=NDC)[:, :, tq * 512:(tq + 1) * 512],
                )

            # ---- persistent activations ----
            qT8 = persist.tile([P, NT], FP8)   # rows: h0 dims 0-63, h1 64-127
            kT8 = persist.tile([P, NT], FP8)
            q8 = persist.tile([64, 2 * NT], FP8)   # [32h+d, (j, t)]; dim = 32j+d of head h
            k8 = persist.tile([64, 2 * NT], FP8)
            qbf = persist.tile([P, B * 512], BF16)  # stripe 0 of each batch
            kbf = persist.tile([P, B * 256], BF16)  # blocks 0-1 of each batch
            # vn blocks padded to 128 cols: [0:64] V dims, col 64 ones
            # (softmax denominators ride along), cols 65:128 zeros (dual-fp8
            # ldweights requires 32-aligned tile strides and M in {32,64,128})
            vn8 = [persist.tile([P, NTT * P], FP8, name=f"vn8_h{h}") for h in range(2)]
            vn32 = [persist.tile([P, B * 2 * P], F32R, name=f"vn32_h{h}") for h in range(2)]
            ctxT = persist.tile([P, NT], BF16)
            # reciprocal landing pad: rows 0 and 64 only; rows 1-63/65-127 stay
            # zero so the hsel65 broadcast matmul contracts clean zeros
            rinv2 = persist.tile([P, 2 * 512], F32)
            nc.gpsimd.memset(rinv2[:], 0.0)

            # ones/zeros pad columns of vn tiles. fp8 memset is legal;
            # f32r memset is not -> stage f32 and copy.
            for h in range(2):
                v8v = vn8[h][:].rearrange("p (t w) -> p t w", w=P)
                nc.gpsimd.memset(v8v[:, :, HD:HD + 1], 1.0)
                nc.gpsimd.memset(v8v[:, :, HD + 1:P], 0.0)
            z64 = const.tile([P, P - HD], F32)
            nc.gpsimd.memset(z64[:], 0.0)
            nc.gpsimd.memset(z64[:, 0:1], 1.0)
            for h in range(2):
                for blk in range(B * 2):
                    nc.vector.tensor_copy(
                        vn32[h][:, blk * P + HD:(blk + 1) * P], z64[:])

            q8v = q8[:].rearrange("p (j t) -> p j t", j=2)
            k8v = k8[:].rearrange("p (j t) -> p j t", j=2)

            # ---------------- A phase: projections ----------------
            def emit_proj_stripe(b, sl):
                s = b * NSPB + sl
                col = s * 512
                for w_sb, outs in (
                    (wq_sb, "q"), (wk_sb, "k"), (wv_sb, "v"),
                ):
                    ps = psA.tile([P, 512], F32, name=f"pp_{s}_{outs}", tag="pp", bufs=1)
                    for c in range(NDC):
                        yield_now = (c % 3 == 2)
                        nc.tensor.matmul(
                            ps[:], w_sb[:, c * P:(c + 1) * P],
                            xt_sb[:, c * NT + col: c * NT + col + 512],
                            start=(c == 0), stop=(c == NDC - 1),
                        )
                        if yield_now:
                            yield
                    if outs == "q":
                        nc.vector.tensor_copy(qT8[:, col:col + 512], ps[:])
                        if sl == 0:
                            nc.vector.tensor_copy(qbf[:, b * 512:(b + 1) * 512], ps[:])
                    elif outs == "k":
                        nc.vector.tensor_copy(kT8[:, col:col + 512], ps[:])
                        if sl == 0:
                            nc.vector.tensor_copy(kbf[:, b * 256:(b + 1) * 256], ps[:, 0:256])
                    else:
                        vstage = vstage_p.tile([P, 512], BF16, name="vstage")
                        nc.vector.tensor_copy(vstage[:], ps[:])
                        vtr = vtr_p.tile([P, 512], BF16, name="vtr")
                        vtrv = vtr[:].rearrange("p (i d) -> p i d", i=4)
                        nc.sync.dma_start(vtrv, vstage[:], transpose=True)
                        # vtr[p, i, d] = v[dim d, token col + 128i + p]
                        tt0 = s * 4
                        for h in range(2):
                            nc.vector.tensor_copy(
                                vn8[h][:].rearrange("p (t w) -> p t w", w=P)[:, tt0:tt0 + 4, 0:HD],
                                vtrv[:, :, h * HD:(h + 1) * HD],
                            )
                        if sl == 0:
                            for h in range(2):
                                nc.vector.tensor_copy(
                                    vn32[h][:].rearrange("p (t w) -> p t w", w=P)[:, b * 2:b * 2 + 2, 0:HD],
                                    vtrv[:, 0:2, h * HD:(h + 1) * HD],
                                )
                    yield

            def emit_repack(b, half):
                cols = slice(b * T + half * 1024, b * T + half * 1024 + 1024)
                for srcT, dst in ((qT8, q8), (kT8, k8)):
                    dstv = dst[:].rearrange("p (j t) -> p j t", j=2)
                    for h in range(2):
                        for j in range(2):
                            nc.sync.dma_start(
                                dstv[32 * h:32 * h + 32, j, cols],
                                srcT[64 * h + 32 * j: 64 * h + 32 * j + 32, cols],
                            )

            # A-work generator: stripes + repacks in dependency order.
            # Progress markers count completed repacks.
            progress = [0]

            def gen_A():
                plan = [
                    ("S", 0, 0), ("S", 0, 1), ("R", 0, 0),
                    ("S", 0, 2), ("S", 0, 3), ("R", 0, 1),
                    ("S", 1, 0), ("S", 1, 1), ("R", 1, 0),
                    ("S", 1, 2), ("S", 1, 3), ("R", 1, 1),
                ]
                for kind, b, i in plan:
                    if kind == "S":
                        yield from emit_proj_stripe(b, i)
                    else:
                        emit_repack(b, i)
                        progress[0] += 1
                        yield

            ga = gen_A()

            def drain_until(n_repacks):
                while progress[0] < n_repacks:
                    if next(ga, "DONE") == "DONE":
                        break

            from collections import deque
            pending = deque()

            pace = [0]

            def filler():
                while pending:
                    try:
                        next(pending[0])
                        return
                    except StopIteration:
                        pending.popleft()
                pace[0] += 1
                next(ga, None)

            # ---------------- B phase: attention + out-proj ----------------
            def cstart(m):
                return 0 if m < 0 else min(128 * m, 384)

            def emit_attn_qs(b, qs):
                qoff = b * T + qs * 512
                npair = 2 * qs + 2
                ctx_h = [
                    psB.tile([P, 512], F32, name=f"ctx_{b}{qs}{h}", tag="ctx", bufs=2)
                    for h in range(2)
                ]
                for prn, pr in enumerate(range(npair)):
                    kb0 = 2 * pr
                    ktt0 = b * KBB + kb0
                    m0 = kb0 - 4 * qs
                    m1 = m0 + 1
                    c_0, c_1 = cstart(m0), cstart(m1)
                    cp = c_0
                    exempt = (qs == 0 and pr == 0) and not SKIP_EXEMPT
                    es = []
                    for h in range(2):
                        ctx = ctx_h[h]
                        sp2 = psB.tile([P, 1024], F32, name=f"sp2_{b}{qs}{h}{pr}", tag="sp2", bufs=2)
                        if exempt:
                            for j, cj in ((0, c_0), (1, c_1)):
                                nc.tensor.matmul(
                                    sp2[:, j * 512 + cj:(j + 1) * 512],
                                    kbf[64 * h:64 * h + 64, b * 256 + j * 128: b * 256 + (j + 1) * 128],
                                    qbf[64 * h:64 * h + 64, b * 512 + cj:(b + 1) * 512],
                                    start=True, stop=True,
                                )
                        else:
                            for j, cj in ((0, c_0), (1, c_1)):
                                ktt = ktt0 + j
                                nc.tensor.matmul(
                                    sp2[:, j * 512 + cj:(j + 1) * 512],
                                    k8v[32 * h:32 * h + 32, :, ktt * P:(ktt + 1) * P],
                                    q8v[32 * h:32 * h + 32, :, qoff + cj:qoff + 512],
                                    start=True, stop=True, perf_mode=DR,
                                )
                        filler()
                        if exempt:
                            e = e32_p.tile([P, 1024], F32R, name="e32")
                        else:
                            e = e8_p.tile([P, 1024], FP8, name="e8")
                        es.append(e)
                        # exp each block over its exact live range
                        if c_0 == c_1:
                            nc.scalar.activation(
                                e[:, c_0:1024], sp2[:, c_0:1024],
                                mybir.ActivationFunctionType.Exp, scale=0.125)
                        else:
                            nc.scalar.activation(
                                e[:, c_0:512], sp2[:, c_0:512],
                                mybir.ActivationFunctionType.Exp, scale=0.125)
                            nc.scalar.activation(
                                e[:, 512 + c_1:1024], sp2[:, 512 + c_1:1024],
                                mybir.ActivationFunctionType.Exp, scale=0.125)
                        for j, m in ((0, m0), (1, m1)):
                            if m >= 0:
                                lo = j * 512 + cp
                                hi = j * 512 + min(128 * m + 128, 512)
                                nc.gpsimd.affine_select(
                                    out=e[:, lo:hi], in_=e[:, lo:hi],
                                    compare_op=mybir.AluOpType.is_ge,
                                    fill=0.0, base=cp - 128 * m,
                                    pattern=[[1, hi - lo]],
                                    channel_multiplier=-1,
                                )
                    st0 = (prn == 0)
                    stN = (prn == npair - 1)
                    for h in range(2):
                        ctx, e = ctx_h[h], es[h]
                        if exempt:
                            l0 = b * 2
                            nc.tensor.matmul(
                                ctx[:, c_0:512], vn32[h][:, l0 * P:(l0 + 1) * P],
                                e[:, c_0:512], start=st0, stop=False,
                            )
                            nc.tensor.matmul(
                                ctx[:, c_1:512], vn32[h][:, (l0 + 1) * P:(l0 + 2) * P],
                                e[:, 512 + c_1:1024], start=False, stop=stN,
                            )
                        else:
                            # masked cols of block j=1 in [cp, c_1) were zeroed
                            # by affine_select; block j=0 live from cp=c_0
                            nc.tensor.matmul(
                                ctx[:, cp:512],
                                vn8[h][:].rearrange("p (t w) -> p t w", w=P)[:, ktt0:ktt0 + 2, :],
                                e[:].rearrange("p (j n) -> p j n", j=2)[:, :, cp:512],
                                start=st0, stop=stN, perf_mode=DR,
                            )
                        filler()

                # ---- epilogue head: free the ctx psum slots quickly ----
                off = ((b * NSPB + qs) % 2) * 512
                with nc.allow_low_precision(reason="f32r softmax denominators"):
                    nc.vector.reciprocal(rinv2[0:1, off:off + 512], ctx_h[0][64:65, :])
                    nc.vector.reciprocal(rinv2[64:65, off:off + 512], ctx_h[1][64:65, :])
                ctxU = []
                for h in range(2):
                    u = r_p.tile([64, 512], BF16, name=f"ctxU{h}", tag=f"ctxU{h}")
                    nc.vector.tensor_copy(u[:], ctx_h[h][0:64, :])
                    ctxU.append(u)

                def tail():
                    rb = psB.tile([P, 512], F32, name=f"rb_{b}{qs}", tag="op", bufs=1)
                    nc.tensor.matmul(rb[:], hsel65[:], rinv2[0:65, off:off + 512],
                                     start=True, stop=True)
                    yield
                    for h in range(2):
                        nc.vector.tensor_mul(
                            ctxT[64 * h:64 * h + 64, qoff:qoff + 512],
                            ctxU[h][:], rb[64 * h:64 * h + 64, :])
                    yield
                    osb = osb_p.tile([P, 4096], BF16, name="osb")
                    for i in range(4):
                        coff = qoff + i * P
                        for half in range(2):
                            op = psB.tile([P, 512], F32, name=f"op_{b}{qs}{i}{half}",
                                          tag="op", bufs=1)
                            nc.tensor.matmul(
                                op[:], ctxT[:, coff:coff + P],
                                wo_sb[:, half * 512:(half + 1) * 512],
                                start=True, stop=True)
                            nc.vector.tensor_copy(
                                osb[:, i * 1024 + half * 512:i * 1024 + (half + 1) * 512],
                                op[:])
                            yield
                    nc.sync.dma_start(
                        out[qoff:qoff + 512, :].rearrange("(i p) d -> p i d", p=P),
                        osb[:].rearrange("p (i d) -> p i d", i=4),
                    )

                pending.append(tail())

            if SKIP_B:
                drain_until(99)
                # dummy out write so the output tensor has a writer
                dumb = osb_p.tile([P, 4096], BF16, name="dumb")
                nc.vector.tensor_copy(dumb[:, 0:512], qbf[:, 0:512])
                nc.sync.dma_start(
                    out[0:512, :].rearrange("(i p) d -> p i d", p=P),
                    dumb[:].rearrange("p (i d) -> p i d", i=4))
            else:
                for b in range(B):
                    for qs in range(NSPB):
                        drain_until(b * 2 + qs // 2 + 1)
                        emit_attn_qs(b, qs)
                drain_until(99)
                while pending:
                    for _ in pending.popleft():
                        pass

    _split_multi_waits(nc)
    return nc


_NC_CACHE = None


def _get_nc():
    global _NC_CACHE
    if _NC_CACHE is None:
        _NC_CACHE = build()
    return _NC_CACHE


def make_in_maps(x, Wq, Wk, Wv, Wo):
    xt = np.ascontiguousarray(
        np.asarray(x, dtype=np.float32).reshape(NT, DIN).T
    ).astype(ml_dtypes.bfloat16)
    in_maps = []
    for c in range(N_CORES):
        sl = slice(c * P, (c + 1) * P)
        in_maps.append({
            "xt": xt,
            "wq": np.ascontiguousarray(np.asarray(Wq, dtype=np.float32)[:, sl]).astype(ml_dtypes.bfloat16),
            "wk": np.ascontiguousarray(np.asarray(Wk, dtype=np.float32)[:, sl]).astype(ml_dtypes.bfloat16),
            "wv": np.ascontiguousarray(np.asarray(Wv, dtype=np.float32)[:, sl]).astype(ml_dtypes.bfloat16),
            "wo": np.ascontiguousarray(np.asarray(Wo, dtype=np.float32)[sl, :]).astype(ml_dtypes.bfloat16),
        })
    return in_maps


def kernel(x, Wq, Wk, Wv, Wo, bo):
    nc = _get_nc()
    in_maps = make_in_maps(x, Wq, Wk, Wv, Wo)
    res = bass_utils.run_bass_kernel_spmd(
        nc, in_maps, core_ids=list(range(N_CORES)), trace=False
    )
    acc = np.zeros((NT, DIN), dtype=np.float64)
    for r in res.results:
        acc += np.asarray(r["out"]).astype(np.float64)
    acc += np.asarray(bo, dtype=np.float64)[None, :]
    return acc.astype(np.float32).reshape(B, T, DIN)


if __name__ == "__main__":
    rng = np.random.default_rng(0)
    x = rng.standard_normal((B, T, DIN)).astype(np.float32)
    Wq = rng.standard_normal((DIN, DIN)).astype(np.float32) * 0.02
    Wk = rng.standard_normal((DIN, DIN)).astype(np.float32) * 0.02
    Wv = rng.standard_normal((DIN, DIN)).astype(np.float32) * 0.02
    Wo = rng.standard_normal((DIN, DIN)).astype(np.float32) * 0.02
    bo = np.zeros(DIN, dtype=np.float32)
    out = kernel(x=x, Wq=Wq, Wk=Wk, Wv=Wv, Wo=Wo, bo=bo)
    print("out", out.shape, out.dtype, float(np.abs(out).max()))


# revision 33
# speedup vs baseline: 1.0599x; 1.0066x over previous
"""Multi-head causal attention (b=2, t=2048, d=1024, H=16, hd=64) on 8 TRN2 cores.

Sharding: tensor-parallel over heads - 2 heads per core. Each core projects
the full token stream against its 128-dim weight slice, runs causal
attention for its 2 heads, and multiplies by its Wo row slice into a
[4096, 1024] bf16 partial that the host sums (+bias).

v2 layout/dtype strategy (per-core):
  * x, weights, out in bf16 (halves DMA traffic); projections in bf16.
  * q/k evicted to fp8e4 and DMA-repacked [64, 2j, t] so QK^T runs as an
    fp8 DoubleRow matmul (2 rows/cycle).
  * exp(scores) written as fp8 into [128, (2 blocks) x 512] tiles; the
    attn*V matmul contracts a PAIR of 128-token k-blocks per instruction
    with DoubleRow (vn pair [128, 2, 65]; the 65th column of ones gives
    softmax denominators for free).
  * numerics guard: causal rows q<512 attend k<256 through a bf16/f32r
    path (fp8 error does not average out on short rows).
  * V transposed via DMA-transpose XBAR instead of PE transposes.
  * projection (A) work for later batches is interleaved into attention
    (B) emission to fill PE dependency stalls.
"""

import sys

for _p in ("/opt/trn_rl_repo",):
    if _p not in sys.path:
        sys.path.insert(0, _p)

import numpy as np
import ml_dtypes

import concourse.bass as bass
import concourse.tile as tile
from concourse import mybir
from concourse import bass_utils

F32 = mybir.dt.float32
F32R = mybir.dt.float32r
BF16 = mybir.dt.bfloat16
FP8 = mybir.dt.float8e4
DR = mybir.MatmulPerfMode.DoubleRow

P = 128          # partitions
B = 2            # batch
T = 2048         # seq len
NT = B * T       # 4096 tokens
DIN = 1024       # model dim
HD = 64          # head dim
NDC = DIN // P   # 8 d_in chunks
NSPB = T // 512  # 4 token stripes per batch
KBB = T // P     # 16 k-blocks per batch
NTT = NT // P    # 32 token tiles
VW = HD + 1      # vn block width (64 dims + ones col)

N_CORES = 8

import os
USE_DR_SCORES = os.environ.get('NO_DR_SCORES', '') == ''
USE_DR_CTX = os.environ.get('NO_DR_CTX', '') == ''
SKIP_B = os.environ.get('SKIP_B', '') != ''
SKIP_EXEMPT = os.environ.get('SKIP_EXEMPT', '') != ''


def _split_multi_waits(nc, max_waits=1):
    """walrus in this container caps sync waits per instruction; spill
    extra waits onto same-engine NoOps inserted right before."""
    uid = 0
    for fn in nc.m.functions:
        for blk in fn.blocks:
            insts = blk.instructions
            new_list = []
            changed = False
            for inst in insts:
                si = inst.sync_info
                ow = list(si.on_wait) if si is not None and si.on_wait else []
                if len(ow) > max_waits:
                    spill, keep = ow[:-max_waits], ow[-max_waits:]
                    for w in spill:
                        nop = mybir.InstNoOp(name=f"I-wsplit-{blk.name}-{uid}", ins=[], outs=[])
                        uid += 1
                        nop.engine = inst.engine
                        nop.sync_info = mybir.SyncInfo(on_wait=[w], on_update=[])
                        new_list.append(nop)
                    inst.sync_info = mybir.SyncInfo(
                        on_wait=keep,
                        on_update=list(si.on_update) if si.on_update else [],
                    )
                    changed = True
                new_list.append(inst)
            if changed:
                insts[:] = new_list


def build():
    nc = bass.Bass("TRN2", target_bir_lowering=False, debug=False, num_devices=N_CORES)
    xt = nc.dram_tensor("xt", [DIN, NT], BF16, kind="ExternalInput").ap()
    wq = nc.dram_tensor("wq", [DIN, P], BF16, kind="ExternalInput").ap()
    wk = nc.dram_tensor("wk", [DIN, P], BF16, kind="ExternalInput").ap()
    wv = nc.dram_tensor("wv", [DIN, P], BF16, kind="ExternalInput").ap()
    wo = nc.dram_tensor("wo", [P, DIN], BF16, kind="ExternalInput").ap()
    out = nc.dram_tensor("out", [NT, DIN], BF16, kind="ExternalOutput").ap()

    with tile.TileContext(nc) as tc:
        with (
            tc.tile_pool(name="const", bufs=1) as const,
            tc.tile_pool(name="persist", bufs=1) as persist,
            tc.tile_pool(name="vstage_p", bufs=4) as vstage_p,
            tc.tile_pool(name="vtr_p", bufs=4) as vtr_p,
            tc.tile_pool(name="e8_p", bufs=6) as e8_p,
            tc.tile_pool(name="e32_p", bufs=2) as e32_p,
            tc.tile_pool(name="r_p", bufs=2) as r_p,
            tc.tile_pool(name="osb_p", bufs=2) as osb_p,
            tc.tile_pool(name="psB", bufs=1, space="PSUM") as psB,
        ):
            # ---- constants / weights ----
            # head-selector for the rb broadcast: row 0 -> partitions 0:64,
            # row 64 -> partitions 64:128 (engine writes are legal at
            # partition bases 0 mod 32 only)
            hsel65 = const.tile([65, P], F32)
            nc.gpsimd.memset(hsel65[:], 0.0)
            nc.gpsimd.memset(hsel65[0:1, 0:64], 1.0)
            nc.gpsimd.memset(hsel65[64:65, 64:128], 1.0)



            wq_sb = const.tile([P, NDC * P], BF16)
            wk_sb = const.tile([P, NDC * P], BF16)
            wv_sb = const.tile([P, NDC * P], BF16)
            for w_dram, w_sb in ((wq, wq_sb), (wk, wk_sb), (wv, wv_sb)):
                nc.sync.dma_start(
                    w_sb[:].rearrange("p (c n) -> p c n", c=NDC),
                    w_dram.rearrange("(c p) n -> p c n", c=NDC),
                )
            wo_sb = const.tile([P, DIN], BF16)
            nc.sync.dma_start(wo_sb[:], wo)

            # x resident in SBUF: [p, (chunk c, token t)]
            xt_sb = const.tile([P, NDC * NT], BF16)
            for tq in range(8):
                nc.sync.dma_start(
                    xt_sb[:].rearrange("p (c n) -> p c n", c=NDC)[:, :, tq * 512:(tq + 1) * 512],
                    xt.rearrange("(c p) n -> p c n", c=NDC)[:, :, tq * 512:(tq + 1) * 512],
                )

            # ---- persistent activations ----
            qT8 = persist.tile([P, NT], FP8)   # rows: h0 dims 0-63, h1 64-127
            kT8 = persist.tile([P, NT], FP8)
            q8 = persist.tile([64, 2 * NT], FP8)   # [32h+d, (j, t)]; dim = 32j+d of head h
            k8 = persist.tile([64, 2 * NT], FP8)
            qbf = persist.tile([P, B * 512], BF16)  # stripe 0 of each batch
            kbf = persist.tile([P, B * 256], BF16)  # blocks 0-1 of each batch
            # vn blocks padded to 128 cols: [0:64] V dims, col 64 ones
            # (softmax denominators ride along), cols 65:128 zeros (dual-fp8
            # ldweights requires 32-aligned tile strides and M in {32,64,128})
            vn8 = [persist.tile([P, NTT * P], FP8, name=f"vn8_h{h}") for h in range(2)]
            vn32 = [persist.tile([P, B * 2 * P], F32R, name=f"vn32_h{h}") for h in range(2)]
            ctxT = persist.tile([P, NT], BF16)
            # reciprocal landing pad: rows 0 and 64 only; rows 1-63/65-127 stay
            # zero so the hsel65 broadcast matmul contracts clean zeros
            rinv2 = persist.tile([P, 2 * 512], F32)
            nc.gpsimd.memset(rinv2[:], 0.0)

            # ones/zeros pad columns of vn tiles. fp8 memset is legal;
            # f32r memset is not -> stage f32 and copy.
            for h in range(2):
                v8v = vn8[h][:].rearrange("p (t w) -> p t w", w=P)
                nc.gpsimd.memset(v8v[:, :, HD:HD + 1], 1.0)
                nc.gpsimd.memset(v8v[:, :, HD + 1:P], 0.0)
            z64 = const.tile([P, P - HD], F32)
            nc.gpsimd.memset(z64[:], 0.0)
            nc.gpsimd.memset(z64[:, 0:1], 1.0)
            for h in range(2):
                for blk in range(B * 2):
                    nc.vector.tensor_copy(
                        vn32[h][:, blk * P + HD:(blk + 1) * P], z64[:])

            q8v = q8[:].rearrange("p (j t) -> p j t", j=2)
            k8v = k8[:].rearrange("p (j t) -> p j t", j=2)

            # ---------------- A phase: projections ----------------
            def emit_proj_stripe(b, sl):
                s = b * NSPB + sl
                col = s * 512
                for w_sb, outs in (
                    (wq_sb, "q"), (wk_sb, "k"), (wv_sb, "v"),
                ):
                    ps = psB.tile([P, 512], F32, name=f"pp_{s}_{outs}", tag="scratch", bufs=2)
                    for c in range(NDC):
                        yield_now = (c % 3 == 2)
                        nc.tensor.matmul(
                            ps[:], w_sb[:, c * P:(c + 1) * P],
                            xt_sb[:, c * NT + col: c * NT + col + 512],
                            start=(c == 0), stop=(c == NDC - 1),
                        )
                        if yield_now:
                            yield
                    if outs == "q":
                        nc.vector.tensor_copy(qT8[:, col:col + 512], ps[:])
                        if sl == 0:
                            nc.vector.tensor_copy(qbf[:, b * 512:(b + 1) * 512], ps[:])
                    elif outs == "k":
                        nc.vector.tensor_copy(kT8[:, col:col + 512], ps[:])
                        if sl == 0:
                            nc.vector.tensor_copy(kbf[:, b * 256:(b + 1) * 256], ps[:, 0:256])
                    else:
                        vstage = vstage_p.tile([P, 512], BF16, name="vstage")
                        nc.vector.tensor_copy(vstage[:], ps[:])
                        vtr = vtr_p.tile([P, 512], BF16, name="vtr")
                        vtrv = vtr[:].rearrange("p (i d) -> p i d", i=4)
                        nc.sync.dma_start(vtrv, vstage[:], transpose=True)
                        # vtr[p, i, d] = v[dim d, token col + 128i + p]
                        tt0 = s * 4
                        for h in range(2):
                            nc.vector.tensor_copy(
                                vn8[h][:].rearrange("p (t w) -> p t w", w=P)[:, tt0:tt0 + 4, 0:HD],
                                vtrv[:, :, h * HD:(h + 1) * HD],
                            )
                        if sl == 0:
                            for h in range(2):
                                nc.vector.tensor_copy(
                                    vn32[h][:].rearrange("p (t w) -> p t w", w=P)[:, b * 2:b * 2 + 2, 0:HD],
                                    vtrv[:, 0:2, h * HD:(h + 1) * HD],
                                )
                    yield

            def emit_repack(b, half):
                cols = slice(b * T + half * 1024, b * T + half * 1024 + 1024)
                for srcT, dst in ((qT8, q8), (kT8, k8)):
                    dstv = dst[:].rearrange("p (j t) -> p j t", j=2)
                    for h in range(2):
                        for j in range(2):
                            nc.sync.dma_start(
                                dstv[32 * h:32 * h + 32, j, cols],
                                srcT[64 * h + 32 * j: 64 * h + 32 * j + 32, cols],
                            )

            # A-work generator: stripes + repacks in dependency order.
            # Progress markers count completed repacks.
            progress = [0]

            def gen_A():
                plan = [
                    ("S", 0, 0), ("S", 0, 1), ("R", 0, 0),
                    ("S", 0, 2), ("S", 0, 3), ("R", 0, 1),
                    ("S", 1, 0), ("S", 1, 1), ("R", 1, 0),
                    ("S", 1, 2), ("S", 1, 3), ("R", 1, 1),
                ]
                for kind, b_, i_ in plan:
                    if kind == "S":
                        yield from emit_proj_stripe(b_, i_)
                    else:
                        emit_repack(b_, i_)
                        progress[0] += 1
                        yield

            ga = gen_A()

            def drain_until(n_repacks):
                while progress[0] < n_repacks:
                    if next(ga, "DONE") == "DONE":
                        break

            from collections import deque
            pending = deque()

            pace = [0]

            def filler():
                while pending:
                    try:
                        next(pending[0])
                        return
                    except StopIteration:
                        pending.popleft()
                pace[0] += 1
                next(ga, None)

            # ---------------- B phase: attention + out-proj ----------------
            def cstart(m):
                return 0 if m < 0 else min(128 * m, 384)

            def emit_attn_qs(b, qs):
                qoff = b * T + qs * 512
                npair = 2 * qs + 2
                ctx_h = [
                    psB.tile([P, 512], F32, name=f"ctx_{b}{qs}{h}", tag="ctx", bufs=2)
                    for h in range(2)
                ]
                prev_ctx = [None]

                def emit_ctx(pr, es, exempt, c_0, c_1, cp, ktt0, st0, stN):
                    for h in range(2):
                        ctx, e = ctx_h[h], es[h]
                        if exempt:
                            l0 = b * 2
                            nc.tensor.matmul(
                                ctx[:, c_0:512], vn32[h][:, l0 * P:(l0 + 1) * P],
                                e[:, c_0:512], start=st0, stop=False,
                            )
                            nc.tensor.matmul(
                                ctx[:, c_1:512], vn32[h][:, (l0 + 1) * P:(l0 + 2) * P],
                                e[:, 512 + c_1:1024], start=False, stop=stN,
                            )
                        else:
                            nc.tensor.matmul(
                                ctx[:, cp:512],
                                vn8[h][:].rearrange("p (t w) -> p t w", w=P)[:, ktt0:ktt0 + 2, :],
                                e[:].rearrange("p (j n) -> p j n", j=2)[:, :, cp:512],
                                start=st0, stop=stN, perf_mode=DR,
                            )
                        filler()

                for prn, pr in enumerate(range(npair)):
                    kb0 = 2 * pr
                    ktt0 = b * KBB + kb0
                    m0 = kb0 - 4 * qs
                    m1 = m0 + 1
                    c_0, c_1 = cstart(m0), cstart(m1)
                    cp = c_0
                    exempt = (qs == 0 and pr == 0) and not SKIP_EXEMPT
                    es = []
                    for h in range(2):
                        sp2 = psB.tile([P, 1024], F32, name=f"sp2_{b}{qs}{h}{pr}", tag="sp2", bufs=2)
                        if exempt:
                            for j, cj in ((0, c_0), (1, c_1)):
                                nc.tensor.matmul(
                                    sp2[:, j * 512 + cj:(j + 1) * 512],
                                    kbf[64 * h:64 * h + 64, b * 256 + j * 128: b * 256 + (j + 1) * 128],
                                    qbf[64 * h:64 * h + 64, b * 512 + cj:(b + 1) * 512],
                                    start=True, stop=True,
                                )
                        else:
                            for j, cj in ((0, c_0), (1, c_1)):
                                ktt = ktt0 + j
                                nc.tensor.matmul(
                                    sp2[:, j * 512 + cj:(j + 1) * 512],
                                    k8v[32 * h:32 * h + 32, :, ktt * P:(ktt + 1) * P],
                                    q8v[32 * h:32 * h + 32, :, qoff + cj:qoff + 512],
                                    start=True, stop=True, perf_mode=DR,
                                )
                        filler()
                        if exempt:
                            e = e32_p.tile([P, 1024], F32R, name="e32")
                        else:
                            e = e8_p.tile([P, 1024], FP8, name="e8")
                        es.append(e)
                        if c_0 == c_1:
                            nc.scalar.activation(
                                e[:, c_0:1024], sp2[:, c_0:1024],
                                mybir.ActivationFunctionType.Exp, scale=0.125)
                        else:
                            nc.scalar.activation(
                                e[:, c_0:512], sp2[:, c_0:512],
                                mybir.ActivationFunctionType.Exp, scale=0.125)
                            nc.scalar.activation(
                                e[:, 512 + c_1:1024], sp2[:, 512 + c_1:1024],
                                mybir.ActivationFunctionType.Exp, scale=0.125)
                        for j, m in ((0, m0), (1, m1)):
                            if m >= 0:
                                lo = j * 512 + cp
                                hi = j * 512 + min(128 * m + 128, 512)
                                nc.gpsimd.affine_select(
                                    out=e[:, lo:hi], in_=e[:, lo:hi],
                                    compare_op=mybir.AluOpType.is_ge,
                                    fill=0.0, base=cp - 128 * m,
                                    pattern=[[1, hi - lo]],
                                    channel_multiplier=-1,
                                )
                    st0 = (prn == 0)
                    stN = (prn == npair - 1)
                    emit_ctx(pr, es, exempt, c_0, c_1, cp, ktt0, st0, stN)

                # ---- epilogue head: free the ctx psum slots quickly ----
                off = ((b * NSPB + qs) % 2) * 512
                with nc.allow_low_precision(reason="f32r softmax denominators"):
                    nc.vector.reciprocal(rinv2[0:1, off:off + 512], ctx_h[0][64:65, :])
                    nc.vector.reciprocal(rinv2[64:65, off:off + 512], ctx_h[1][64:65, :])
                ctxU = []
                for h in range(2):
                    u = r_p.tile([64, 512], BF16, name=f"ctxU{h}", tag=f"ctxU{h}")
                    nc.vector.tensor_copy(u[:], ctx_h[h][0:64, :])
                    ctxU.append(u)

                def tail():
                    rb = psB.tile([P, 512], F32, name=f"rb_{b}{qs}", tag="scratch", bufs=2)
                    nc.tensor.matmul(rb[:], hsel65[:], rinv2[0:65, off:off + 512],
                                     start=True, stop=True)
                    yield
                    for h in range(2):
                        nc.vector.tensor_mul(
                            ctxT[64 * h:64 * h + 64, qoff:qoff + 512],
                            ctxU[h][:], rb[64 * h:64 * h + 64, :])
                    yield
                    osb = osb_p.tile([P, 4096], BF16, name="osb")
                    for i in range(4):
                        coff = qoff + i * P
                        for half in range(2):
                            op = psB.tile([P, 512], F32, name=f"op_{b}{qs}{i}{half}",
                                          tag="scratch", bufs=2)
                            nc.tensor.matmul(
                                op[:], ctxT[:, coff:coff + P],
                                wo_sb[:, half * 512:(half + 1) * 512],
                                start=True, stop=True)
                            dst = osb[:, i * 1024 + half * 512:i * 1024 + (half + 1) * 512]
                            if half == 1 and i % 2 == 0:
                                nc.scalar.copy(dst, op[:])
                            else:
                                nc.vector.tensor_copy(dst, op[:])
                            yield
                    nc.sync.dma_start(
                        out[qoff:qoff + 512, :].rearrange("(i p) d -> p i d", p=P),
                        osb[:].rearrange("p (i d) -> p i d", i=4),
                    )

                pending.append(tail())

            if SKIP_B:
                drain_until(99)
                # dummy out write so the output tensor has a writer
                dumb = osb_p.tile([P, 4096], BF16, name="dumb")
                nc.vector.tensor_copy(dumb[:, 0:512], qbf[:, 0:512])
                nc.sync.dma_start(
                    out[0:512, :].rearrange("(i p) d -> p i d", p=P),
                    dumb[:].rearrange("p (i d) -> p i d", i=4))
            else:
                for b in range(B):
                    for qs in range(NSPB):
                        drain_until(b * 2 + qs // 2 + 1)
                        emit_attn_qs(b, qs)
                drain_until(99)
                while pending:
                    for _ in pending.popleft():
                        pass

    _split_multi_waits(nc)
    return nc


_NC_CACHE = None


def _get_nc():
    global _NC_CACHE
    if _NC_CACHE is None:
        _NC_CACHE = build()
    return _NC_CACHE


def make_in_maps(x, Wq, Wk, Wv, Wo):
    xt = np.ascontiguousarray(
        np.asarray(x, dtype=np.float32).reshape(NT, DIN).T
    ).astype(ml_dtypes.bfloat16)
    in_maps = []
    for c in range(N_CORES):
        sl = slice(c * P, (c + 1) * P)
        in_maps.append({
            "xt": xt,
            "wq": np.ascontiguousarray(np.asarray(Wq, dtype=np.float32)[:, sl]).astype(ml_dtypes.bfloat16),
            "wk": np.ascontiguousarray(np.asarray(Wk, dtype=np.float32)[:, sl]).astype(ml_dtypes.bfloat16),
            "wv": np.ascontiguousarray(np.asarray(Wv, dtype=np.float32)[:, sl]).astype(ml_dtypes.bfloat16),
            "wo": np.ascontiguousarray(np.asarray(Wo, dtype=np.float32)[sl, :]).astype(ml_dtypes.bfloat16),
        })
    return in_maps


def kernel(x, Wq, Wk, Wv, Wo, bo):
    nc = _get_nc()
    in_maps = make_in_maps(x, Wq, Wk, Wv, Wo)
    res = bass_utils.run_bass_kernel_spmd(
        nc, in_maps, core_ids=list(range(N_CORES)), trace=False
    )
    acc = np.zeros((NT, DIN), dtype=np.float64)
    for r in res.results:
        acc += np.asarray(r["out"]).astype(np.float64)
    acc += np.asarray(bo, dtype=np.float64)[None, :]
    return acc.astype(np.float32).reshape(B, T, DIN)


if __name__ == "__main__":
    rng = np.random.default_rng(0)
    x = rng.standard_normal((B, T, DIN)).astype(np.float32)
    Wq = rng.standard_normal((DIN, DIN)).astype(np.float32) * 0.02
    Wk = rng.standard_normal((DIN, DIN)).astype(np.float32) * 0.02
    Wv = rng.standard_normal((DIN, DIN)).astype(np.float32) * 0.02
    Wo = rng.standard_normal((DIN, DIN)).astype(np.float32) * 0.02
    bo = np.zeros(DIN, dtype=np.float32)
    out = kernel(x=x, Wq=Wq, Wk=Wk, Wv=Wv, Wo=Wo, bo=bo)
    print("out", out.shape, out.dtype, float(np.abs(out).max()))


# revision 54
# speedup vs baseline: 1.1009x; 1.0387x over previous
"""Multi-head causal attention (b=2, t=2048, d=1024, H=16, hd=64) on 8 TRN2 cores.

Sharding: tensor-parallel over heads - 2 heads per core. Each core projects
the full token stream against its 128-dim weight slice, runs causal
attention for its 2 heads, and multiplies by its Wo row slice into a
[4096, 1024] bf16 partial that the host sums (+bias).

v2 layout/dtype strategy (per-core):
  * x, weights, out in bf16 (halves DMA traffic); projections in bf16.
  * q/k evicted to fp8e4 and DMA-repacked [64, 2j, t] so QK^T runs as an
    fp8 DoubleRow matmul (2 rows/cycle).
  * exp(scores) written as fp8 into [128, (2 blocks) x 512] tiles; the
    attn*V matmul contracts a PAIR of 128-token k-blocks per instruction
    with DoubleRow (vn pair [128, 2, 65]; the 65th column of ones gives
    softmax denominators for free).
  * numerics guard: causal rows q<512 attend k<256 through a bf16/f32r
    path (fp8 error does not average out on short rows).
  * V transposed via DMA-transpose XBAR instead of PE transposes.
  * projection (A) work for later batches is interleaved into attention
    (B) emission to fill PE dependency stalls.
"""

import sys

for _p in ("/opt/trn_rl_repo",):
    if _p not in sys.path:
        sys.path.insert(0, _p)

import numpy as np
import ml_dtypes

import concourse.bass as bass
import concourse.tile as tile
from concourse import mybir
from concourse import bass_utils

F32 = mybir.dt.float32
F32R = mybir.dt.float32r
BF16 = mybir.dt.bfloat16
FP8 = mybir.dt.float8e4
DR = mybir.MatmulPerfMode.DoubleRow

P = 128          # partitions
B = 2            # batch
T = 2048         # seq len
NT = B * T       # 4096 tokens
DIN = 1024       # model dim
HD = 64          # head dim
NDC = DIN // P   # 8 d_in chunks
NSPB = T // 512  # 4 token stripes per batch
KBB = T // P     # 16 k-blocks per batch
NTT = NT // P    # 32 token tiles
VW = HD + 1      # vn block width (64 dims + ones col)

N_CORES = 8

import os
USE_DR_SCORES = os.environ.get('NO_DR_SCORES', '') == ''
USE_DR_CTX = os.environ.get('NO_DR_CTX', '') == ''
SKIP_B = os.environ.get('SKIP_B', '') != ''
SKIP_EXEMPT = os.environ.get('SKIP_EXEMPT', '') != ''


def _split_multi_waits(nc, max_waits=1):
    """walrus in this container caps sync waits per instruction; spill
    extra waits onto same-engine NoOps inserted right before."""
    uid = 0
    for fn in nc.m.functions:
        for blk in fn.blocks:
            insts = blk.instructions
            new_list = []
            changed = False
            for inst in insts:
                si = inst.sync_info
                ow = list(si.on_wait) if si is not None and si.on_wait else []
                if len(ow) > max_waits:
                    spill, keep = ow[:-max_waits], ow[-max_waits:]
                    for w in spill:
                        nop = mybir.InstNoOp(name=f"I-wsplit-{blk.name}-{uid}", ins=[], outs=[])
                        uid += 1
                        nop.engine = inst.engine
                        nop.sync_info = mybir.SyncInfo(on_wait=[w], on_update=[])
                        new_list.append(nop)
                    inst.sync_info = mybir.SyncInfo(
                        on_wait=keep,
                        on_update=list(si.on_update) if si.on_update else [],
                    )
                    changed = True
                new_list.append(inst)
            if changed:
                insts[:] = new_list


def build():
    nc = bass.Bass("TRN2", target_bir_lowering=False, debug=False, num_devices=N_CORES)
    xt = nc.dram_tensor("xt", [DIN, NT], BF16, kind="ExternalInput").ap()
    wq = nc.dram_tensor("wq", [DIN, P], BF16, kind="ExternalInput").ap()
    wk = nc.dram_tensor("wk", [DIN, P], BF16, kind="ExternalInput").ap()
    wv = nc.dram_tensor("wv", [DIN, P], BF16, kind="ExternalInput").ap()
    wo = nc.dram_tensor("wo", [P, DIN], BF16, kind="ExternalInput").ap()
    out = nc.dram_tensor("out", [NT, DIN], BF16, kind="ExternalOutput").ap()

    with tile.TileContext(nc) as tc:
        with (
            tc.tile_pool(name="const", bufs=1) as const,
            tc.tile_pool(name="persist", bufs=1) as persist,
            tc.tile_pool(name="vstage_p", bufs=4) as vstage_p,
            tc.tile_pool(name="vtr_p", bufs=4) as vtr_p,
            tc.tile_pool(name="e8_p", bufs=8) as e8_p,
            tc.tile_pool(name="e32_p", bufs=2) as e32_p,
            tc.tile_pool(name="r_p", bufs=2) as r_p,
            tc.tile_pool(name="osb_p", bufs=3) as osb_p,
            tc.tile_pool(name="psB", bufs=1, space="PSUM") as psB,
        ):
            # ---- constants / weights ----
            # head-selector for the rb broadcast: row 0 -> partitions 0:64,
            # row 64 -> partitions 64:128 (engine writes are legal at
            # partition bases 0 mod 32 only)
            hsel65 = const.tile([65, P], F32)
            nc.gpsimd.memset(hsel65[:], 0.0)
            nc.gpsimd.memset(hsel65[0:1, 0:64], 1.0)
            nc.gpsimd.memset(hsel65[64:65, 64:128], 1.0)



            wq_sb = const.tile([P, NDC * P], BF16)
            wk_sb = const.tile([P, NDC * P], BF16)
            wv_sb = const.tile([P, NDC * P], BF16)
            for w_dram, w_sb in ((wq, wq_sb), (wk, wk_sb), (wv, wv_sb)):
                nc.sync.dma_start(
                    w_sb[:].rearrange("p (c n) -> p c n", c=NDC),
                    w_dram.rearrange("(c p) n -> p c n", c=NDC),
                )
            wo_sb = const.tile([P, DIN], BF16)
            nc.sync.dma_start(wo_sb[:], wo)

            # x resident in SBUF: [p, (chunk c, token t)]; loaded stripe-by-
            # stripe inside the A generator so repack/transpose DMAs aren't
            # stuck behind the full 8 MB load on the DMA engines
            xt_sb = const.tile([P, NDC * NT], BF16)

            def load_x_stripe(s):
                nc.sync.dma_start(
                    xt_sb[:].rearrange("p (c n) -> p c n", c=NDC)[:, :, s * 512:(s + 1) * 512],
                    xt.rearrange("(c p) n -> p c n", c=NDC)[:, :, s * 512:(s + 1) * 512],
                )

            # ---- persistent activations ----
            qT8 = persist.tile([P, NT], FP8)   # rows: h0 dims 0-63, h1 64-127
            kT8 = persist.tile([P, NT], FP8)
            q8 = persist.tile([64, 2 * NT], FP8)   # [32h+d, (j, t)]; dim = 32j+d of head h
            k8 = persist.tile([64, 2 * NT], FP8)
            qbf = persist.tile([P, B * 512], BF16)  # stripe 0 of each batch
            kbf = persist.tile([P, B * 256], BF16)  # blocks 0-1 of each batch
            # vn blocks padded to 128 cols: [0:64] V dims, col 64 ones
            # (softmax denominators ride along), cols 65:128 zeros (dual-fp8
            # ldweights requires 32-aligned tile strides and M in {32,64,128})
            vn8 = [persist.tile([P, NTT * P], FP8, name=f"vn8_h{h}") for h in range(2)]
            vn32 = [persist.tile([P, B * 2 * P], F32R, name=f"vn32_h{h}") for h in range(2)]
            ctxT = persist.tile([P, NT], BF16)
            # reciprocal landing pad: rows 0 and 64 only; rows 1-63/65-127 stay
            # zero so the hsel65 broadcast matmul contracts clean zeros
            rinv2 = persist.tile([P, 2 * 512], F32)
            nc.gpsimd.memset(rinv2[:], 0.0)

            # ones/zeros pad columns of vn tiles. fp8 memset is legal;
            # f32r memset is not -> stage f32 and copy.
            for h in range(2):
                v8v = vn8[h][:].rearrange("p (t w) -> p t w", w=P)
                nc.gpsimd.memset(v8v[:, :, HD:HD + 1], 1.0)
                nc.gpsimd.memset(v8v[:, :, HD + 1:P], 0.0)
            z64 = const.tile([P, P - HD], F32)
            nc.gpsimd.memset(z64[:], 0.0)
            nc.gpsimd.memset(z64[:, 0:1], 1.0)
            for h in range(2):
                for blk in range(B * 2):
                    nc.vector.tensor_copy(
                        vn32[h][:, blk * P + HD:(blk + 1) * P], z64[:])

            q8v = q8[:].rearrange("p (j t) -> p j t", j=2)
            k8v = k8[:].rearrange("p (j t) -> p j t", j=2)

            # ---------------- A phase: projections ----------------
            def emit_proj_stripe(b, sl):
                s = b * NSPB + sl
                col = s * 512
                for w_sb, outs in (
                    (wq_sb, "q"), (wk_sb, "k"), (wv_sb, "v"),
                ):
                    ps = psB.tile([P, 512], F32, name=f"pp_{s}_{outs}", tag="scratch", bufs=2)
                    for c in range(NDC):
                        yield_now = (c % 3 == 2)
                        nc.tensor.matmul(
                            ps[:], w_sb[:, c * P:(c + 1) * P],
                            xt_sb[:, c * NT + col: c * NT + col + 512],
                            start=(c == 0), stop=(c == NDC - 1),
                        )
                        if yield_now:
                            yield
                    if outs == "q":
                        nc.vector.tensor_copy(qT8[:, col:col + 512], ps[:])
                        if sl == 0:
                            nc.vector.tensor_copy(qbf[:, b * 512:(b + 1) * 512], ps[:])
                    elif outs == "k":
                        nc.vector.tensor_copy(kT8[:, col:col + 512], ps[:])
                        if sl == 0:
                            nc.vector.tensor_copy(kbf[:, b * 256:(b + 1) * 256], ps[:, 0:256])
                    else:
                        vstage = vstage_p.tile([P, 512], BF16, name="vstage")
                        nc.vector.tensor_copy(vstage[:], ps[:])
                        vtr = vtr_p.tile([P, 512], BF16, name="vtr")
                        vtrv = vtr[:].rearrange("p (i d) -> p i d", i=4)
                        nc.sync.dma_start(vtrv, vstage[:], transpose=True)
                        # vtr[p, i, d] = v[dim d, token col + 128i + p]
                        tt0 = s * 4
                        for h in range(2):
                            nc.gpsimd.tensor_copy(
                                vn8[h][:].rearrange("p (t w) -> p t w", w=P)[:, tt0:tt0 + 4, 0:HD],
                                vtrv[:, :, h * HD:(h + 1) * HD],
                            )
                        if sl == 0:
                            for h in range(2):
                                nc.gpsimd.tensor_copy(
                                    vn32[h][:].rearrange("p (t w) -> p t w", w=P)[:, b * 2:b * 2 + 2, 0:HD],
                                    vtrv[:, 0:2, h * HD:(h + 1) * HD],
                                )
                    yield

            def emit_repack(b, half):
                cols = slice(b * T + half * 1024, b * T + half * 1024 + 1024)
                for srcT, dst in ((qT8, q8), (kT8, k8)):
                    dstv = dst[:].rearrange("p (j t) -> p j t", j=2)
                    for h in range(2):
                        for j in range(2):
                            nc.sync.dma_start(
                                dstv[32 * h:32 * h + 32, j, cols],
                                srcT[64 * h + 32 * j: 64 * h + 32 * j + 32, cols],
                            )

            # A-work generator: stripes + repacks in dependency order.
            # Progress markers count completed repacks.
            progress = [0]

            def gen_A():
                plan = [
                    ("S", 0, 0), ("S", 0, 1), ("R", 0, 0),
                    ("S", 0, 2), ("S", 0, 3), ("R", 0, 1),
                    ("S", 1, 0), ("S", 1, 1), ("R", 1, 0),
                    ("S", 1, 2), ("S", 1, 3), ("R", 1, 1),
                ]
                for kind, b_, i_ in plan:
                    if kind == "S":
                        yield from emit_proj_stripe(b_, i_)
                    else:
                        emit_repack(b_, i_)
                        progress[0] += 1
                        yield

            for s_ in range(8):
                load_x_stripe(s_)
            # PE p-state warm-up: the cost model runs matmuls at 1/3 clock
            # until the PE has been continuously busy ~3us. Burn dummy
            # matmuls during the x-load window so the first projection
            # stripes run at full clock.
            wdum = psB.tile([P, 512], F32, name="wdum", tag="scratch", bufs=2)
            for _ in range(16):
                nc.tensor.matmul(wdum[0:1, 0:128], hsel65[0:1, 0:1],
                                 hsel65[0:1, 0:128], start=True, stop=True)
            ga = gen_A()

            def drain_until(n_repacks):
                while progress[0] < n_repacks:
                    if next(ga, "DONE") == "DONE":
                        break

            from collections import deque
            pending = deque()

            pace = [0]

            def filler():
                while pending:
                    try:
                        next(pending[0])
                        return
                    except StopIteration:
                        pending.popleft()
                pace[0] += 1
                next(ga, None)

            # ---------------- B phase: attention + out-proj ----------------
            def cstart(m):
                return 0 if m < 0 else min(128 * m, 384)

            def emit_attn_qs(b, qs):
                qoff = b * T + qs * 512
                npair = 2 * qs + 2
                ctx_h = [
                    psB.tile([P, 512], F32, name=f"ctx_{b}{qs}{h}", tag="ctx", bufs=2)
                    for h in range(2)
                ]
                def emit_ctx(pr, es, exempt, c_0, c_1, cp, ktt0, st0, stN):
                    for h in range(2):
                        ctx, e = ctx_h[h], es[h]
                        if exempt:
                            l0 = b * 2
                            nc.tensor.matmul(
                                ctx[:, c_0:512], vn32[h][:, l0 * P:(l0 + 1) * P],
                                e[:, c_0:512], start=st0, stop=False,
                            )
                            nc.tensor.matmul(
                                ctx[:, c_1:512], vn32[h][:, (l0 + 1) * P:(l0 + 2) * P],
                                e[:, 512 + c_1:1024], start=False, stop=stN,
                            )
                        else:
                            nc.tensor.matmul(
                                ctx[:, cp:512],
                                vn8[h][:].rearrange("p (t w) -> p t w", w=P)[:, ktt0:ktt0 + 2, :],
                                e[:].rearrange("p (j n) -> p j n", j=2)[:, :, cp:512],
                                start=st0, stop=stN, perf_mode=DR,
                            )
                        filler()

                for prn, pr in enumerate(range(npair)):
                    kb0 = 2 * pr
                    ktt0 = b * KBB + kb0
                    m0 = kb0 - 4 * qs
                    m1 = m0 + 1
                    c_0, c_1 = cstart(m0), cstart(m1)
                    cp = c_0
                    exempt = (qs == 0 and pr == 0) and not SKIP_EXEMPT
                    es = []
                    for h in range(2):
                        sp2 = psB.tile([P, 1024], F32, name=f"sp2_{b}{qs}{h}{pr}", tag="sp2", bufs=2)
                        if exempt:
                            for j, cj in ((0, c_0), (1, c_1)):
                                nc.tensor.matmul(
                                    sp2[:, j * 512 + cj:(j + 1) * 512],
                                    kbf[64 * h:64 * h + 64, b * 256 + j * 128: b * 256 + (j + 1) * 128],
                                    qbf[64 * h:64 * h + 64, b * 512 + cj:(b + 1) * 512],
                                    start=True, stop=True,
                                )
                        else:
                            for j, cj in ((0, c_0), (1, c_1)):
                                ktt = ktt0 + j
                                nc.tensor.matmul(
                                    sp2[:, j * 512 + cj:(j + 1) * 512],
                                    k8v[32 * h:32 * h + 32, :, ktt * P:(ktt + 1) * P],
                                    q8v[32 * h:32 * h + 32, :, qoff + cj:qoff + 512],
                                    start=True, stop=True, perf_mode=DR,
                                )
                        filler()
                        if exempt:
                            e = e32_p.tile([P, 1024], F32R, name="e32")
                        else:
                            e = e8_p.tile([P, 1024], FP8, name="e8")
                        es.append(e)
                        if c_0 == c_1:
                            nc.scalar.activation(
                                e[:, c_0:1024], sp2[:, c_0:1024],
                                mybir.ActivationFunctionType.Exp, scale=0.125)
                        else:
                            nc.scalar.activation(
                                e[:, c_0:512], sp2[:, c_0:512],
                                mybir.ActivationFunctionType.Exp, scale=0.125)
                            nc.scalar.activation(
                                e[:, 512 + c_1:1024], sp2[:, 512 + c_1:1024],
                                mybir.ActivationFunctionType.Exp, scale=0.125)
                        for j, m in ((0, m0), (1, m1)):
                            if m >= 0:
                                lo = j * 512 + cp
                                hi = j * 512 + min(128 * m + 128, 512)
                                nc.gpsimd.affine_select(
                                    out=e[:, lo:hi], in_=e[:, lo:hi],
                                    compare_op=mybir.AluOpType.is_ge,
                                    fill=0.0, base=cp - 128 * m,
                                    pattern=[[1, hi - lo]],
                                    channel_multiplier=-1,
                                )
                    st0 = (prn == 0)
                    stN = (prn == npair - 1)
                    emit_ctx(pr, es, exempt, c_0, c_1, cp, ktt0, st0, stN)

                # ---- epilogue head: free the ctx psum slots quickly ----
                off = ((b * NSPB + qs) % 2) * 512
                with nc.allow_low_precision(reason="f32r softmax denominators"):
                    nc.vector.reciprocal(rinv2[0:1, off:off + 512], ctx_h[0][64:65, :])
                    nc.vector.reciprocal(rinv2[64:65, off:off + 512], ctx_h[1][64:65, :])
                ctxU = []
                for h in range(2):
                    u = r_p.tile([64, 512], BF16, name=f"ctxU{h}", tag=f"ctxU{h}")
                    nc.vector.tensor_copy(u[:], ctx_h[h][0:64, :])
                    ctxU.append(u)

                last_qs = (b == B - 1 and qs == NSPB - 1)

                def tail():
                    rb = psB.tile([P, 512], F32, name=f"rb_{b}{qs}", tag="scratch", bufs=2)
                    nc.tensor.matmul(rb[:], hsel65[:], rinv2[0:65, off:off + 512],
                                     start=True, stop=True)
                    yield
                    for h in range(2):
                        nc.vector.tensor_mul(
                            ctxT[64 * h:64 * h + 64, qoff:qoff + 512],
                            ctxU[h][:], rb[64 * h:64 * h + 64, :])
                    yield
                    osb = osb_p.tile([P, 4096], BF16, name="osb")
                    for i in range(4):
                        coff = qoff + i * P
                        for half in range(2):
                            op = psB.tile([P, 512], F32, name=f"op_{b}{qs}{i}{half}",
                                          tag="scratch", bufs=2)
                            nc.tensor.matmul(
                                op[:], ctxT[:, coff:coff + P],
                                wo_sb[:, half * 512:(half + 1) * 512],
                                start=True, stop=True)
                            dst = osb[:, i * 1024 + half * 512:i * 1024 + (half + 1) * 512]
                            if last_qs and half == 1:
                                nc.scalar.copy(dst, op[:])
                            else:
                                nc.vector.tensor_copy(dst, op[:])
                            yield
                    nc.sync.dma_start(
                        out[qoff:qoff + 512, :].rearrange("(i p) d -> p i d", p=P),
                        osb[:].rearrange("p (i d) -> p i d", i=4),
                    )

                pending.append(tail())

            if SKIP_B:
                drain_until(99)
                # dummy out write so the output tensor has a writer
                dumb = osb_p.tile([P, 4096], BF16, name="dumb")
                nc.vector.tensor_copy(dumb[:, 0:512], qbf[:, 0:512])
                nc.sync.dma_start(
                    out[0:512, :].rearrange("(i p) d -> p i d", p=P),
                    dumb[:].rearrange("p (i d) -> p i d", i=4))
            else:
                for b in range(B):
                    for qs in range(NSPB):
                        drain_until(b * 2 + qs // 2 + 1)
                        emit_attn_qs(b, qs)
                drain_until(99)
                while pending:
                    try:
                        next(pending[0])
                        pending.rotate(-1)
                    except StopIteration:
                        pending.popleft()

    _split_multi_waits(nc)
    return nc


_NC_CACHE = None


def _get_nc():
    global _NC_CACHE
    if _NC_CACHE is None:
        _NC_CACHE = build()
    return _NC_CACHE


def make_in_maps(x, Wq, Wk, Wv, Wo):
    xt = np.ascontiguousarray(
        np.asarray(x, dtype=np.float32).reshape(NT, DIN).T
    ).astype(ml_dtypes.bfloat16)
    in_maps = []
    for c in range(N_CORES):
        sl = slice(c * P, (c + 1) * P)
        in_maps.append({
            "xt": xt,
            "wq": np.ascontiguousarray(np.asarray(Wq, dtype=np.float32)[:, sl]).astype(ml_dtypes.bfloat16),
            "wk": np.ascontiguousarray(np.asarray(Wk, dtype=np.float32)[:, sl]).astype(ml_dtypes.bfloat16),
            "wv": np.ascontiguousarray(np.asarray(Wv, dtype=np.float32)[:, sl]).astype(ml_dtypes.bfloat16),
            "wo": np.ascontiguousarray(np.asarray(Wo, dtype=np.float32)[sl, :]).astype(ml_dtypes.bfloat16),
        })
    return in_maps


def kernel(x, Wq, Wk, Wv, Wo, bo):
    nc = _get_nc()
    in_maps = make_in_maps(x, Wq, Wk, Wv, Wo)
    res = bass_utils.run_bass_kernel_spmd(
        nc, in_maps, core_ids=list(range(N_CORES)), trace=False
    )
    acc = np.zeros((NT, DIN), dtype=np.float64)
    for r in res.results:
        acc += np.asarray(r["out"]).astype(np.float64)
    acc += np.asarray(bo, dtype=np.float64)[None, :]
    return acc.astype(np.float32).reshape(B, T, DIN)


if __name__ == "__main__":
    rng = np.random.default_rng(0)
    x = rng.standard_normal((B, T, DIN)).astype(np.float32)
    Wq = rng.standard_normal((DIN, DIN)).astype(np.float32) * 0.02
    Wk = rng.standard_normal((DIN, DIN)).astype(np.float32) * 0.02
    Wv = rng.standard_normal((DIN, DIN)).astype(np.float32) * 0.02
    Wo = rng.standard_normal((DIN, DIN)).astype(np.float32) * 0.02
    bo = np.zeros(DIN, dtype=np.float32)
    out = kernel(x=x, Wq=Wq, Wk=Wk, Wv=Wv, Wo=Wo, bo=bo)
    print("out", out.shape, out.dtype, float(np.abs(out).max()))


# revision 57
# speedup vs baseline: 1.1140x; 1.0118x over previous
"""Multi-head causal attention (b=2, t=2048, d=1024, H=16, hd=64) on 8 TRN2 cores.

Sharding: tensor-parallel over heads - 2 heads per core. Each core projects
the full token stream against its 128-dim weight slice, runs causal
attention for its 2 heads, and multiplies by its Wo row slice into a
[4096, 1024] bf16 partial that the host sums (+bias).

v2 layout/dtype strategy (per-core):
  * x, weights, out in bf16 (halves DMA traffic); projections in bf16.
  * q/k evicted to fp8e4 and DMA-repacked [64, 2j, t] so QK^T runs as an
    fp8 DoubleRow matmul (2 rows/cycle).
  * exp(scores) written as fp8 into [128, (2 blocks) x 512] tiles; the
    attn*V matmul contracts a PAIR of 128-token k-blocks per instruction
    with DoubleRow (vn pair [128, 2, 65]; the 65th column of ones gives
    softmax denominators for free).
  * numerics guard: causal rows q<512 attend k<256 through a bf16/f32r
    path (fp8 error does not average out on short rows).
  * V transposed via DMA-transpose XBAR instead of PE transposes.
  * projection (A) work for later batches is interleaved into attention
    (B) emission to fill PE dependency stalls.
"""

import sys

for _p in ("/opt/trn_rl_repo",):
    if _p not in sys.path:
        sys.path.insert(0, _p)

import numpy as np
import ml_dtypes

import concourse.bass as bass
import concourse.tile as tile
from concourse import mybir
from concourse import bass_utils

F32 = mybir.dt.float32
F32R = mybir.dt.float32r
BF16 = mybir.dt.bfloat16
FP8 = mybir.dt.float8e4
DR = mybir.MatmulPerfMode.DoubleRow

P = 128          # partitions
B = 2            # batch
T = 2048         # seq len
NT = B * T       # 4096 tokens
DIN = 1024       # model dim
HD = 64          # head dim
NDC = DIN // P   # 8 d_in chunks
NSPB = T // 512  # 4 token stripes per batch
KBB = T // P     # 16 k-blocks per batch
NTT = NT // P    # 32 token tiles
VW = HD + 1      # vn block width (64 dims + ones col)

N_CORES = 8

import os
USE_DR_SCORES = os.environ.get('NO_DR_SCORES', '') == ''
USE_DR_CTX = os.environ.get('NO_DR_CTX', '') == ''
SKIP_B = os.environ.get('SKIP_B', '') != ''
SKIP_EXEMPT = os.environ.get('SKIP_EXEMPT', '') != ''


def _split_multi_waits(nc, max_waits=1):
    """walrus in this container caps sync waits per instruction; spill
    extra waits onto same-engine NoOps inserted right before."""
    uid = 0
    for fn in nc.m.functions:
        for blk in fn.blocks:
            insts = blk.instructions
            new_list = []
            changed = False
            for inst in insts:
                si = inst.sync_info
                ow = list(si.on_wait) if si is not None and si.on_wait else []
                if len(ow) > max_waits:
                    spill, keep = ow[:-max_waits], ow[-max_waits:]
                    for w in spill:
                        nop = mybir.InstNoOp(name=f"I-wsplit-{blk.name}-{uid}", ins=[], outs=[])
                        uid += 1
                        nop.engine = inst.engine
                        nop.sync_info = mybir.SyncInfo(on_wait=[w], on_update=[])
                        new_list.append(nop)
                    inst.sync_info = mybir.SyncInfo(
                        on_wait=keep,
                        on_update=list(si.on_update) if si.on_update else [],
                    )
                    changed = True
                new_list.append(inst)
            if changed:
                insts[:] = new_list


def build():
    nc = bass.Bass("TRN2", target_bir_lowering=False, debug=False, num_devices=N_CORES)
    xt = nc.dram_tensor("xt", [DIN, NT], BF16, kind="ExternalInput").ap()
    wq = nc.dram_tensor("wq", [DIN, P], BF16, kind="ExternalInput").ap()
    wk = nc.dram_tensor("wk", [DIN, P], BF16, kind="ExternalInput").ap()
    wv = nc.dram_tensor("wv", [DIN, P], BF16, kind="ExternalInput").ap()
    wo = nc.dram_tensor("wo", [P, DIN], BF16, kind="ExternalInput").ap()
    out = nc.dram_tensor("out", [NT, DIN], BF16, kind="ExternalOutput").ap()

    with tile.TileContext(nc) as tc:
        with (
            tc.tile_pool(name="const", bufs=1) as const,
            tc.tile_pool(name="persist", bufs=1) as persist,
            tc.tile_pool(name="vstage_p", bufs=4) as vstage_p,
            tc.tile_pool(name="vtr_p", bufs=4) as vtr_p,
            tc.tile_pool(name="e8_p", bufs=8) as e8_p,
            tc.tile_pool(name="e32_p", bufs=2) as e32_p,
            tc.tile_pool(name="r_p", bufs=2) as r_p,
            tc.tile_pool(name="osb_p", bufs=3) as osb_p,
            tc.tile_pool(name="psB", bufs=1, space="PSUM") as psB,
        ):
            # ---- constants / weights ----
            # head-selector for the rb broadcast: row 0 -> partitions 0:64,
            # row 64 -> partitions 64:128 (engine writes are legal at
            # partition bases 0 mod 32 only)
            hsel65 = const.tile([65, P], F32)
            nc.gpsimd.memset(hsel65[:], 0.0)
            nc.gpsimd.memset(hsel65[0:1, 0:64], 1.0)
            nc.gpsimd.memset(hsel65[64:65, 64:128], 1.0)



            wq_sb = const.tile([P, NDC * P], BF16)
            wk_sb = const.tile([P, NDC * P], BF16)
            wv_sb = const.tile([P, NDC * P], BF16)
            for w_dram, w_sb in ((wq, wq_sb), (wk, wk_sb), (wv, wv_sb)):
                nc.sync.dma_start(
                    w_sb[:].rearrange("p (c n) -> p c n", c=NDC),
                    w_dram.rearrange("(c p) n -> p c n", c=NDC),
                )
            wo_sb = const.tile([P, DIN], BF16)
            nc.sync.dma_start(wo_sb[:], wo)

            # x resident in SBUF: [p, (chunk c, token t)]; loaded stripe-by-
            # stripe inside the A generator so repack/transpose DMAs aren't
            # stuck behind the full 8 MB load on the DMA engines
            xt_sb = const.tile([P, NDC * NT], BF16)

            def load_x_stripe(s):
                nc.sync.dma_start(
                    xt_sb[:].rearrange("p (c n) -> p c n", c=NDC)[:, :, s * 512:(s + 1) * 512],
                    xt.rearrange("(c p) n -> p c n", c=NDC)[:, :, s * 512:(s + 1) * 512],
                )

            # ---- persistent activations ----
            qT8 = persist.tile([P, NT], FP8)   # rows: h0 dims 0-63, h1 64-127
            kT8 = persist.tile([P, NT], FP8)
            q8 = persist.tile([64, 2 * NT], FP8)   # [32h+d, (j, t)]; dim = 32j+d of head h
            k8 = persist.tile([64, 2 * NT], FP8)
            qbf = persist.tile([P, B * 512], BF16)  # stripe 0 of each batch
            kbf = persist.tile([P, B * 256], BF16)  # blocks 0-1 of each batch
            # vn blocks padded to 128 cols: [0:64] V dims, col 64 ones
            # (softmax denominators ride along), cols 65:128 zeros (dual-fp8
            # ldweights requires 32-aligned tile strides and M in {32,64,128})
            vn8 = [persist.tile([P, NTT * P], FP8, name=f"vn8_h{h}") for h in range(2)]
            vn32 = [persist.tile([P, B * 2 * P], F32R, name=f"vn32_h{h}") for h in range(2)]
            ctxT = persist.tile([P, NT], BF16)
            # reciprocal landing pad: rows 0 and 64 only; rows 1-63/65-127 stay
            # zero so the hsel65 broadcast matmul contracts clean zeros
            rinv2 = persist.tile([P, 2 * 512], F32)
            nc.gpsimd.memset(rinv2[:], 0.0)

            # ones/zeros pad columns of vn tiles. fp8 memset is legal;
            # f32r memset is not -> stage f32 and copy.
            for h in range(2):
                v8v = vn8[h][:].rearrange("p (t w) -> p t w", w=P)
                nc.gpsimd.memset(v8v[:, :, HD:HD + 1], 1.0)
                nc.gpsimd.memset(v8v[:, :, HD + 1:P], 0.0)
            z64 = const.tile([P, P - HD], F32)
            nc.gpsimd.memset(z64[:], 0.0)
            nc.gpsimd.memset(z64[:, 0:1], 1.0)
            for h in range(2):
                for blk in range(B * 2):
                    nc.vector.tensor_copy(
                        vn32[h][:, blk * P + HD:(blk + 1) * P], z64[:])

            q8v = q8[:].rearrange("p (j t) -> p j t", j=2)
            k8v = k8[:].rearrange("p (j t) -> p j t", j=2)

            # ---------------- A phase: projections ----------------
            def emit_proj_stripe(b, sl):
                s = b * NSPB + sl
                col = s * 512
                for w_sb, outs in (
                    (wq_sb, "q"), (wk_sb, "k"), (wv_sb, "v"),
                ):
                    ps = psB.tile([P, 512], F32, name=f"pp_{s}_{outs}", tag="scratch", bufs=2)
                    for c in range(NDC):
                        yield_now = (c % 3 == 2)
                        nc.tensor.matmul(
                            ps[:], w_sb[:, c * P:(c + 1) * P],
                            xt_sb[:, c * NT + col: c * NT + col + 512],
                            start=(c == 0), stop=(c == NDC - 1),
                        )
                        if yield_now:
                            yield
                    if outs == "q":
                        nc.vector.tensor_copy(qT8[:, col:col + 512], ps[:])
                        if sl == 0:
                            nc.vector.tensor_copy(qbf[:, b * 512:(b + 1) * 512], ps[:])
                    elif outs == "k":
                        nc.vector.tensor_copy(kT8[:, col:col + 512], ps[:])
                        if sl == 0:
                            nc.vector.tensor_copy(kbf[:, b * 256:(b + 1) * 256], ps[:, 0:256])
                    else:
                        vstage = vstage_p.tile([P, 512], BF16, name="vstage")
                        nc.vector.tensor_copy(vstage[:], ps[:])
                        vtr = vtr_p.tile([P, 512], BF16, name="vtr")
                        vtrv = vtr[:].rearrange("p (i d) -> p i d", i=4)
                        nc.sync.dma_start(vtrv, vstage[:], transpose=True)
                        # vtr[p, i, d] = v[dim d, token col + 128i + p]
                        tt0 = s * 4
                        for h in range(2):
                            nc.gpsimd.tensor_copy(
                                vn8[h][:].rearrange("p (t w) -> p t w", w=P)[:, tt0:tt0 + 4, 0:HD],
                                vtrv[:, :, h * HD:(h + 1) * HD],
                            )
                        if sl == 0:
                            for h in range(2):
                                nc.gpsimd.tensor_copy(
                                    vn32[h][:].rearrange("p (t w) -> p t w", w=P)[:, b * 2:b * 2 + 2, 0:HD],
                                    vtrv[:, 0:2, h * HD:(h + 1) * HD],
                                )
                    yield

            def emit_repack(b, half):
                cols = slice(b * T + half * 1024, b * T + half * 1024 + 1024)
                for srcT, dst in ((qT8, q8), (kT8, k8)):
                    dstv = dst[:].rearrange("p (j t) -> p j t", j=2)
                    for h in range(2):
                        for j in range(2):
                            nc.sync.dma_start(
                                dstv[32 * h:32 * h + 32, j, cols],
                                srcT[64 * h + 32 * j: 64 * h + 32 * j + 32, cols],
                            )

            # A-work generator: stripes + repacks in dependency order.
            # Progress markers count completed repacks.
            progress = [0]

            def gen_A():
                plan = [
                    ("S", 0, 0), ("S", 0, 1), ("R", 0, 0),
                    ("S", 0, 2), ("S", 0, 3), ("R", 0, 1),
                    ("S", 1, 0), ("S", 1, 1), ("R", 1, 0),
                    ("S", 1, 2), ("S", 1, 3), ("R", 1, 1),
                ]
                for kind, b_, i_ in plan:
                    if kind == "S":
                        yield from emit_proj_stripe(b_, i_)
                    else:
                        emit_repack(b_, i_)
                        progress[0] += 1
                        yield

            for s_ in range(8):
                load_x_stripe(s_)
            # PE p-state warm-up: the cost model runs matmuls at 1/3 clock
            # until the PE has been continuously busy ~3us. Burn dummy
            # matmuls during the x-load window so the first projection
            # stripes run at full clock.
            wdum = psB.tile([P, 512], F32, name="wdum", tag="scratch", bufs=2)
            for _ in range(16):
                nc.tensor.matmul(wdum[0:1, 0:128], hsel65[0:1, 0:1],
                                 hsel65[0:1, 0:128], start=True, stop=True)
            ga = gen_A()

            def drain_until(n_repacks):
                while progress[0] < n_repacks:
                    if next(ga, "DONE") == "DONE":
                        break

            from collections import deque
            pending = deque()

            pace = [0]

            def filler():
                while pending:
                    try:
                        next(pending[0])
                        return
                    except StopIteration:
                        pending.popleft()
                pace[0] += 1
                next(ga, None)

            # ---------------- B phase: attention + out-proj ----------------
            def cstart(m):
                return 0 if m < 0 else min(128 * m, 384)

            def emit_attn_qs(b, qs):
                qoff = b * T + qs * 512
                npair = 2 * qs + 2
                ctx_h = [
                    psB.tile([P, 512], F32, name=f"ctx_{b}{qs}{h}", tag="ctx", bufs=2)
                    for h in range(2)
                ]
                def emit_ctx(pr, es, exempt, c_0, c_1, cp, ktt0, st0, stN):
                    for h in range(2):
                        ctx, e = ctx_h[h], es[h]
                        if exempt:
                            l0 = b * 2
                            nc.tensor.matmul(
                                ctx[:, c_0:512], vn32[h][:, l0 * P:(l0 + 1) * P],
                                e[:, c_0:512], start=st0, stop=False,
                            )
                            nc.tensor.matmul(
                                ctx[:, c_1:512], vn32[h][:, (l0 + 1) * P:(l0 + 2) * P],
                                e[:, 512 + c_1:1024], start=False, stop=stN,
                            )
                        else:
                            nc.tensor.matmul(
                                ctx[:, cp:512],
                                vn8[h][:].rearrange("p (t w) -> p t w", w=P)[:, ktt0:ktt0 + 2, :],
                                e[:].rearrange("p (j n) -> p j n", j=2)[:, :, cp:512],
                                start=st0, stop=stN, perf_mode=DR,
                            )
                        filler()

                for prn, pr in enumerate(range(npair)):
                    kb0 = 2 * pr
                    ktt0 = b * KBB + kb0
                    m0 = kb0 - 4 * qs
                    m1 = m0 + 1
                    c_0, c_1 = cstart(m0), cstart(m1)
                    cp = c_0
                    exempt = (qs == 0 and pr == 0) and not SKIP_EXEMPT
                    es = []
                    for h in range(2):
                        sp2 = psB.tile([P, 1024], F32, name=f"sp2_{b}{qs}{h}{pr}", tag="sp2", bufs=2)
                        if exempt:
                            for j, cj in ((0, c_0), (1, c_1)):
                                nc.tensor.matmul(
                                    sp2[:, j * 512 + cj:(j + 1) * 512],
                                    kbf[64 * h:64 * h + 64, b * 256 + j * 128: b * 256 + (j + 1) * 128],
                                    qbf[64 * h:64 * h + 64, b * 512 + cj:(b + 1) * 512],
                                    start=True, stop=True,
                                )
                        else:
                            for j, cj in ((0, c_0), (1, c_1)):
                                ktt = ktt0 + j
                                nc.tensor.matmul(
                                    sp2[:, j * 512 + cj:(j + 1) * 512],
                                    k8v[32 * h:32 * h + 32, :, ktt * P:(ktt + 1) * P],
                                    q8v[32 * h:32 * h + 32, :, qoff + cj:qoff + 512],
                                    start=True, stop=True, perf_mode=DR,
                                )
                        filler()
                        if exempt:
                            e = e32_p.tile([P, 1024], F32R, name="e32")
                        else:
                            e = e8_p.tile([P, 1024], FP8, name="e8")
                        es.append(e)
                        if c_0 == c_1:
                            nc.scalar.activation(
                                e[:, c_0:1024], sp2[:, c_0:1024],
                                mybir.ActivationFunctionType.Exp, scale=0.125)
                        else:
                            nc.scalar.activation(
                                e[:, c_0:512], sp2[:, c_0:512],
                                mybir.ActivationFunctionType.Exp, scale=0.125)
                            nc.scalar.activation(
                                e[:, 512 + c_1:1024], sp2[:, 512 + c_1:1024],
                                mybir.ActivationFunctionType.Exp, scale=0.125)
                        for j, m in ((0, m0), (1, m1)):
                            if m >= 0:
                                lo = j * 512 + cp
                                hi = j * 512 + min(128 * m + 128, 512)
                                nc.gpsimd.affine_select(
                                    out=e[:, lo:hi], in_=e[:, lo:hi],
                                    compare_op=mybir.AluOpType.is_ge,
                                    fill=0.0, base=cp - 128 * m,
                                    pattern=[[1, hi - lo]],
                                    channel_multiplier=-1,
                                )
                    st0 = (prn == 0)
                    stN = (prn == npair - 1)
                    emit_ctx(pr, es, exempt, c_0, c_1, cp, ktt0, st0, stN)

                # ---- epilogue head: free the ctx psum slots quickly ----
                off = ((b * NSPB + qs) % 2) * 512
                with nc.allow_low_precision(reason="f32r softmax denominators"):
                    nc.vector.reciprocal(rinv2[0:1, off:off + 512], ctx_h[0][64:65, :])
                    nc.vector.reciprocal(rinv2[64:65, off:off + 512], ctx_h[1][64:65, :])
                ctxU = []
                for h in range(2):
                    u = r_p.tile([64, 512], BF16, name=f"ctxU{h}", tag=f"ctxU{h}")
                    nc.vector.tensor_copy(u[:], ctx_h[h][0:64, :])
                    ctxU.append(u)

                last_qs = (b == B - 1 and qs == NSPB - 1)

                def tail():
                    rb = psB.tile([P, 512], F32, name=f"rb_{b}{qs}", tag="scratch", bufs=2)
                    nc.tensor.matmul(rb[:], hsel65[:], rinv2[0:65, off:off + 512],
                                     start=True, stop=True)
                    yield
                    for h in range(2):
                        nc.vector.tensor_mul(
                            ctxT[64 * h:64 * h + 64, qoff:qoff + 512],
                            ctxU[h][:], rb[64 * h:64 * h + 64, :])
                    yield
                    osb = osb_p.tile([P, 4096], BF16, name="osb")
                    for i in range(4):
                        coff = qoff + i * P
                        for half in range(2):
                            op = psB.tile([P, 512], F32, name=f"op_{b}{qs}{i}{half}",
                                          tag="scratch", bufs=2)
                            nc.tensor.matmul(
                                op[:], ctxT[:, coff:coff + P],
                                wo_sb[:, half * 512:(half + 1) * 512],
                                start=True, stop=True)
                            dst = osb[:, i * 1024 + half * 512:i * 1024 + (half + 1) * 512]
                            if last_qs and half == 1:
                                nc.scalar.copy(dst, op[:])
                            else:
                                nc.vector.tensor_copy(dst, op[:])
                            yield
                        nc.sync.dma_start(
                            out[coff:coff + P, :],
                            osb[:, i * 1024:(i + 1) * 1024],
                        )

                pending.append(tail())

            if SKIP_B:
                drain_until(99)
                # dummy out write so the output tensor has a writer
                dumb = osb_p.tile([P, 4096], BF16, name="dumb")
                nc.vector.tensor_copy(dumb[:, 0:512], qbf[:, 0:512])
                nc.sync.dma_start(
                    out[0:512, :].rearrange("(i p) d -> p i d", p=P),
                    dumb[:].rearrange("p (i d) -> p i d", i=4))
            else:
                for b in range(B):
                    for qs in range(NSPB):
                        drain_until(b * 2 + qs // 2 + 1)
                        emit_attn_qs(b, qs)
                drain_until(99)
                while pending:
                    try:
                        next(pending[0])
                        pending.rotate(-1)
                    except StopIteration:
                        pending.popleft()

    _split_multi_waits(nc)
    return nc


_NC_CACHE = None


def _get_nc():
    global _NC_CACHE
    if _NC_CACHE is None:
        _NC_CACHE = build()
    return _NC_CACHE


def make_in_maps(x, Wq, Wk, Wv, Wo):
    xt = np.ascontiguousarray(
        np.asarray(x, dtype=np.float32).reshape(NT, DIN).T
    ).astype(ml_dtypes.bfloat16)
    in_maps = []
    for c in range(N_CORES):
        sl = slice(c * P, (c + 1) * P)
        in_maps.append({
            "xt": xt,
            "wq": np.ascontiguousarray(np.asarray(Wq, dtype=np.float32)[:, sl]).astype(ml_dtypes.bfloat16),
            "wk": np.ascontiguousarray(np.asarray(Wk, dtype=np.float32)[:, sl]).astype(ml_dtypes.bfloat16),
            "wv": np.ascontiguousarray(np.asarray(Wv, dtype=np.float32)[:, sl]).astype(ml_dtypes.bfloat16),
            "wo": np.ascontiguousarray(np.asarray(Wo, dtype=np.float32)[sl, :]).astype(ml_dtypes.bfloat16),
        })
    return in_maps


def kernel(x, Wq, Wk, Wv, Wo, bo):
    nc = _get_nc()
    in_maps = make_in_maps(x, Wq, Wk, Wv, Wo)
    res = bass_utils.run_bass_kernel_spmd(
        nc, in_maps, core_ids=list(range(N_CORES)), trace=False
    )
    acc = np.zeros((NT, DIN), dtype=np.float64)
    for r in res.results:
        acc += np.asarray(r["out"]).astype(np.float64)
    acc += np.asarray(bo, dtype=np.float64)[None, :]
    return acc.astype(np.float32).reshape(B, T, DIN)


if __name__ == "__main__":
    rng = np.random.default_rng(0)
    x = rng.standard_normal((B, T, DIN)).astype(np.float32)
    Wq = rng.standard_normal((DIN, DIN)).astype(np.float32) * 0.02
    Wk = rng.standard_normal((DIN, DIN)).astype(np.float32) * 0.02
    Wv = rng.standard_normal((DIN, DIN)).astype(np.float32) * 0.02
    Wo = rng.standard_normal((DIN, DIN)).astype(np.float32) * 0.02
    bo = np.zeros(DIN, dtype=np.float32)
    out = kernel(x=x, Wq=Wq, Wk=Wk, Wv=Wv, Wo=Wo, bo=bo)
    print("out", out.shape, out.dtype, float(np.abs(out).max()))
